# revision 11
# baseline (speedup 1.0000x reference)
"""Trainium2 Bass kernel for nn_DecoderLayer (B=4,S=2048,D=1024,H=16,FF=4096).

Sharding: 16 heads / 8 cores = 2 heads per core (tensor/head parallel) for
attention; ReduceScatter (fp16) of the head-summed attention output;
token-parallel LN+FFN on each core's 1/8 token shard; host concatenates.

Numerics: host supplies X^T in bf16 and fp8(e4m3); Q/K projections bf16
(2-head-packed stationary, biases via ACT per-partition bias); V projection
and P@V run fp8 DoubleRow (2x contraction per instruction) with exp shifted
by -3.5 so softmax numerators fit fp8 range; Wv is scaled x16 on host so its
values sit in fp8 normal range (descaled at V eviction); softmax/LN in fp32;
FFN matmuls bf16; head outputs accumulated in DRAM as fp16.
"""
import numpy as np
import ml_dtypes
from contextlib import ExitStack

import concourse.bass as bass
import concourse.tile as tile
from concourse import bacc, mybir

dt = mybir.dt
F32 = dt.float32
F16 = dt.float16
BF16 = dt.bfloat16
FP8 = dt.float8e4
AF = mybir.ActivationFunctionType
ALU = mybir.AluOpType
AX = mybir.AxisListType
DR = mybir.MatmulPerfMode.DoubleRow

KD = 64
EPS = 1e-5
NEG = -30000.0
ESHIFT = 3.5          # exp(score - ESHIFT) keeps numerators in fp8 range
WVS = 16.0            # host scales Wv by x16 into fp8 normal range

CFG_MAIN = dict(B=4, S=2048, D=1024, FF=4096, ncores=8, HPC=2)


def build_nc(B, S, D, FF, ncores, HPC):
    DC = D // 128          # d chunks
    CP = DC // 2           # d-chunk pairs (DoubleRow)
    TB = S // 128          # t blocks per batch
    IC = S // 256          # i chunks (256 queries) per batch
    EC = D // 512          # 512-wide e chunks
    TOKC = S // 512
    shard = B * S // ncores
    SB = shard // 128
    FB = FF // 128
    HALVES = max(1, shard // 512)
    TPH = shard // HALVES  # tokens per FFN half
    TBH = TPH // 128
    GF = 4                 # fb group size for y accumulation
    SPB = S // 512         # 512-token slices per batch
    NS = B * S // 512      # total slices
    assert NS % ncores == 0 and HALVES == NS // ncores

    nc = bacc.Bacc("TRN2", target_bir_lowering=False, debug=False,
                   enable_asserts=False, num_devices=ncores)

    # ---- DRAM I/O ----
    xt_d = nc.dram_tensor("xtb", [D, B * S], BF16, kind="ExternalInput").ap()
    xt8_d = nc.dram_tensor("xt8", [D, B * S], FP8, kind="ExternalInput").ap()
    xs_d = nc.dram_tensor("xs", [shard, D], F32, kind="ExternalInput").ap()
    wqk_d = nc.dram_tensor("wqk", [128, 2 * DC * 128], BF16,
                           kind="ExternalInput").ap()
    bqk_d = nc.dram_tensor("bqk", [1, 2 * 128], BF16, kind="ExternalInput").ap()
    wv8_d = nc.dram_tensor("wv8", [HPC, 128, DC * D], FP8,
                           kind="ExternalInput").ap()
    bv_d = nc.dram_tensor("bv", [1, HPC * D], BF16, kind="ExternalInput").ap()
    bvs_d = nc.dram_tensor("bvs", [128, D], F32, kind="ExternalInput").ap()
    padb_d = nc.dram_tensor("padb", [B, 128, TB], F32, kind="ExternalInput").ap()
    q1m_d = nc.dram_tensor("q1m", [128, SB], F32, kind="ExternalInput").ap()
    qp_d = nc.dram_tensor("qp", [128, SB], F32, kind="ExternalInput").ap()
    cm_d = nc.dram_tensor("cmask", [128, 128], F32, kind="ExternalInput").ap()
    id_d = nc.dram_tensor("ident", [128, 128], F32, kind="ExternalInput").ap()
    w1_d = nc.dram_tensor("w1s", [FB, 128, D], BF16, kind="ExternalInput").ap()
    w2_d = nc.dram_tensor("w2", [FF, D], BF16, kind="ExternalInput").ap()
    b1t_d = nc.dram_tensor("b1t", [128, FB], F32, kind="ExternalInput").ap()
    b2b_d = nc.dram_tensor("b2b", [128, D], F32, kind="ExternalInput").ap()
    ln1w_d = nc.dram_tensor("ln1w", [128, D], F32, kind="ExternalInput").ap()
    ln1b_d = nc.dram_tensor("ln1b", [128, D], F32, kind="ExternalInput").ap()
    ln2w_d = nc.dram_tensor("ln2w", [128, D], F32, kind="ExternalInput").ap()
    ln2b_d = nc.dram_tensor("ln2b", [128, D], F32, kind="ExternalInput").ap()
    out_d = nc.dram_tensor("out", [shard, D], F32, kind="ExternalOutput").ap()

    with tile.TileContext(nc) as tc, ExitStack() as ctx0:
        pbig = ctx0.enter_context(tc.tile_pool(name="pbig", bufs=2, space="PSUM"))
        pmed = ctx0.enter_context(tc.tile_pool(name="pmed", bufs=2, space="PSUM"))
        pdn = ctx0.enter_context(tc.tile_pool(name="pdn", bufs=2, space="PSUM"))
        dramp = ctx0.enter_context(tc.tile_pool(name="dram", bufs=1, space="DRAM"))
        consts = ctx0.enter_context(tc.tile_pool(name="const", bufs=1))
        smalls = ctx0.enter_context(tc.tile_pool(name="smalls", bufs=6))

        o_exts = [dramp.tile([(NS // HALVES) * 513, D], F16,
                             name=f"oext{rp}")
                  for rp in range(HALVES)]
        rs_outs = [dramp.tile([513, D], F16, name=f"rsout{rp}")
                   for rp in range(HALVES)]

        # ---- constants ----
        ident = consts.tile([128, 128], F32, tag="ident")
        nc.sync.dma_start(ident[:], id_d[:])
        cmask = consts.tile([128, 128], F32, tag="cmask")
        nc.sync.dma_start(cmask[:], cm_d[:])
        ones_bf = consts.tile([1, 512], BF16, tag="ones_bf")
        nc.vector.memset(ones_bf[:], 1.0)
        ones8 = consts.tile([128, 32], FP8, tag="ones8")
        nc.vector.memset(ones8[:], 1.0)
        ones8_3 = ones8[:].rearrange("p (j u) -> p j u", u=16)[:, :, 0:1]
        # =================== attention phase ===================
        with ExitStack() as actx:
            xtp = actx.enter_context(tc.tile_pool(name="xt", bufs=2))
            xt8p = actx.enter_context(tc.tile_pool(name="xt8", bufs=2))
            wvp = actx.enter_context(tc.tile_pool(name="wv", bufs=2))
            vp = actx.enter_context(tc.tile_pool(name="v", bufs=2))
            vrp = actx.enter_context(tc.tile_pool(name="vr", bufs=2))
            qkp = actx.enter_context(tc.tile_pool(name="qkt", bufs=2))
            ep = actx.enter_context(tc.tile_pool(name="e", bufs=6))
            osbp = actx.enter_context(tc.tile_pool(name="osb", bufs=4))
            padp = actx.enter_context(tc.tile_pool(name="pad", bufs=3))
            aconp = actx.enter_context(tc.tile_pool(name="acon", bufs=1))

            wqk = aconp.tile([128, 2 * DC * 128], BF16, tag="wqk")
            nc.sync.dma_start(wqk[:], wqk_d[:])
            wqk4 = wqk[:].rearrange("p (w c m) -> p w c m", w=2, c=DC)
            bqk = aconp.tile([1, 2 * 128], BF16, tag="bqk")
            nc.sync.dma_start(bqk[:], bqk_d[:])
            bv = aconp.tile([1, HPC * D], BF16, tag="bv")
            nc.sync.dma_start(bv[:], bv_d[:])

            for b in range(B):
                # ---- load X^T tiles (bf16 for QK, fp8 for V) ----
                xt = xtp.tile([128, DC * S], BF16, tag="xt")
                xt8 = xt8p.tile([128, DC * S], FP8, tag="xt8")
                for c in range(DC):
                    nc.sync.dma_start(
                        xt[:, c * S:(c + 1) * S],
                        xt_d[c * 128:(c + 1) * 128, b * S:(b + 1) * S])
                    nc.sync.dma_start(
                        xt8[:, c * S:(c + 1) * S],
                        xt8_d[c * 128:(c + 1) * 128, b * S:(b + 1) * S])
                xt8_3 = xt8[:].rearrange("p (c t) -> p c t", c=DC)
                # column sums of X (for fully-padded-row vmean)
                xsum = smalls.tile([128, DC], F32, tag="xsum")
                for c in range(DC):
                    nc.vector.tensor_reduce(
                        xsum[:, c:c + 1], xt[:, c * S:(c + 1) * S],
                        AX.X, ALU.add)
                xsum8 = smalls.tile([128, DC], FP8, tag="xsum8")
                nc.vector.tensor_copy(out=xsum8[:], in_=xsum[:])

                padt = padp.tile([128, TB], F32, tag="pad")
                nc.sync.dma_start(padt[:], padb_d[b])

                # ---- Q/K projection, both heads packed (M=128) ----
                qkt2 = qkp.tile([128, 2 * S], BF16, tag="qkt")
                for w in range(2):
                    for c4 in range(TOKC):
                        ps = pmed.tile([128, 512], F32, tag="med")
                        for c in range(DC):
                            nc.tensor.matmul(
                                ps[:], wqk4[:, w, c, :],
                                xt[:, c * S + c4 * 512:c * S + c4 * 512 + 512],
                                start=(c == 0), stop=False)
                        nc.tensor.matmul(
                            ps[:], bqk[0:1, w * 128:w * 128 + 128],
                            ones_bf[0:1, :], start=False, stop=True)
                        off = w * S + c4 * 512
                        nc.scalar.activation(
                            qkt2[:, off:off + 512], ps[:], AF.Copy,
                            scale=(0.125 if w == 0 else 1.0))

                # ---- V projection per head (fp8 DoubleRow) + vmean ----
                v8s = []
                for h in range(HPC):
                    wv8 = wvp.tile([128, DC * D], FP8, tag="wv")
                    nc.sync.dma_start(wv8[:], wv8_d[h])
                    wv8_3 = wv8[:].rearrange("p (c e) -> p c e", c=DC)
                    v8 = vp.tile([128, TB * D], FP8, tag="v", name=f"v8_{h}")
                    for tb in range(TB):
                        vps = pbig.tile([128, D], F32, tag="big")
                        for c in range(DC):
                            for ec in range(EC):
                                nc.tensor.matmul(
                                    vps[:, ec * 512:ec * 512 + 512],
                                    xt8[:, c * S + tb * 128:
                                        c * S + tb * 128 + 128],
                                    wv8[:, c * D + ec * 512:
                                        c * D + ec * 512 + 512],
                                    start=(c == 0), stop=(c == DC - 1))
                        nc.scalar.activation(v8[:, tb * D:tb * D + D], vps[:],
                                             AF.Copy, scale=1.0 / WVS)

                    # vmean (for fully-padded rows), via xsum
                    vmps = [pmed.tile([1, 512], F32, tag="med",
                                      name=f"vmps{ec}")
                            for ec in range(EC)]
                    for ec in range(EC):
                        for c in range(DC):
                            nc.tensor.matmul(
                                vmps[ec][:], xsum8[:, c:c + 1],
                                wv8[:, c * D + ec * 512:c * D + ec * 512 + 512],
                                start=(c == 0), stop=(c == DC - 1))
                    vrow = vrp.tile([1, D], F16, tag="vrow")
                    for ec in range(EC):
                        nc.vector.scalar_tensor_tensor(
                            vrow[0:1, ec * 512:ec * 512 + 512], vmps[ec][:],
                            1.0 / (WVS * S),
                            bv[0:1, h * D + ec * 512:h * D + ec * 512 + 512],
                            ALU.mult, ALU.add)
                    spp = NS // HALVES
                    for k in range(SPB):
                        sl = b * SPB + k
                        vr = (sl % spp) * 513 + 512
                        nc.gpsimd.dma_start(
                            o_exts[sl // spp][vr:vr + 1, :], vrow[0:1, :],
                            accum_op=(ALU.bypass if h == 0 else ALU.add))
                    v8s.append(v8)

                # ---- attention per head (scores bf16, P@V fp8 DoubleRow) ----
                for h in range(HPC):
                    hb = h * 64
                    for ic in range(IC):
                        ops = [pbig.tile([128, D], F32, tag="big",
                                         name=f"ops{s}")
                               for s in range(2)]
                        dnp = [pdn.tile([128, 1], F32, tag="dn",
                                        name=f"dnp{s}")
                               for s in range(2)]
                        ntb = 2 * ic + 2

                        def mk_e(tb):
                            st = pmed.tile([128, 256], F32, tag="med",
                                           name=f"st{tb}")
                            nc.tensor.matmul(
                                st[:],
                                qkt2[hb:hb + 64,
                                     S + tb * 128:S + tb * 128 + 128],
                                qkt2[hb:hb + 64, ic * 256:ic * 256 + 256],
                                start=True, stop=True)
                            if tb >= 2 * ic:
                                off = (tb - 2 * ic) * 128
                                nc.vector.tensor_add(
                                    st[:, off:off + 128],
                                    st[:, off:off + 128], cmask[:])
                            e8 = ep.tile([128, 256], FP8, tag="e",
                                         name=f"e{tb}")
                            nc.scalar.activation(e8[:], st[:], AF.Exp,
                                                 bias=padt[:, tb:tb + 1],
                                                 scale=1.0)
                            return e8

                        epipe = {0: mk_e(0)}
                        if ntb > 1:
                            epipe[1] = mk_e(1)
                        for tb in range(ntb):
                            e8 = epipe.pop(tb)
                            if tb + 2 < ntb:
                                epipe[tb + 2] = mk_e(tb + 2)
                            for s in range(2):
                                ib = 2 * ic + s
                                if tb > ib:
                                    continue
                                el = e8[:, s * 128:s * 128 + 128]
                                for ec in range(EC):
                                    nc.tensor.matmul(
                                        ops[s][:, ec * 512:ec * 512 + 512],
                                        el,
                                        v8s[h][:, tb * D + ec * 512:
                                               tb * D + ec * 512 + 512],
                                        start=(tb == 0), stop=(tb == ib))
                                nc.tensor.matmul(
                                    dnp[s][:], el, ones8[:, 0:1],
                                    start=(tb == 0), stop=(tb == ib))
                        for s in range(2):
                            dsb = smalls.tile([128, 1], F32, tag="dsb")
                            nc.vector.tensor_scalar_add(dsb[:], dnp[s][:],
                                                        1e-30)
                            rd = smalls.tile([128, 1], F32, tag="rd")
                            nc.vector.reciprocal(rd[:], dsb[:])
                            osb = osbp.tile([128, D], F16, tag="osb")
                            nc.scalar.activation(osb[:], ops[s][:], AF.Copy,
                                                 scale=rd[:])
                            nc.gpsimd.dma_start(
                                _oext_rows(o_exts, NS // HALVES, b, ic, s, S),
                                osb[:],
                                accum_op=(ALU.bypass if h == 0 else ALU.add))

                # trigger the RS phase whose batches just completed; the
                # collective runs on the CC cores and overlaps later batches
                bpp = B // HALVES
                if (b + 1) % bpp == 0:
                    rp = (b + 1) // bpp - 1
                    nc.gpsimd.collective_compute(
                        "ReduceScatter", ALU.add,
                        replica_groups=[list(range(ncores))],
                        ins=[o_exts[rp].opt()], outs=[rs_outs[rp].opt()])

        # =================== LN + FFN on the token shard ===================
        with ExitStack() as fctx:
            fcon = fctx.enter_context(tc.tile_pool(name="fcon", bufs=1))
            stg = fctx.enter_context(tc.tile_pool(name="stg", bufs=6))
            x1p = fctx.enter_context(tc.tile_pool(name="x1", bufs=TBH + 2))
            x1tp = fctx.enter_context(tc.tile_pool(name="x1t", bufs=2))
            htp = fctx.enter_context(tc.tile_pool(name="hts", bufs=1))
            w1p = fctx.enter_context(tc.tile_pool(name="w1s", bufs=6))
            w2p = fctx.enter_context(tc.tile_pool(name="w2s", bufs=GF + 2))
            ysp = fctx.enter_context(tc.tile_pool(name="ysb", bufs=2 * TBH))
            rsp = fctx.enter_context(tc.tile_pool(name="rsb", bufs=TBH + 2))

            b1t = fcon.tile([128, FB], F32, tag="b1t")
            nc.sync.dma_start(b1t[:], b1t_d[:])
            b2b = fcon.tile([128, D], F32, tag="b2b")
            nc.sync.dma_start(b2b[:], b2b_d[:])
            bvs = fcon.tile([128, D], F32, tag="bvs")
            nc.sync.dma_start(bvs[:], bvs_d[:])
            lnw = []
            for nm, dd in [("ln1w", ln1w_d), ("ln1b", ln1b_d),
                           ("ln2w", ln2w_d), ("ln2b", ln2b_d)]:
                t = fcon.tile([128, D], F32, tag=nm)
                nc.sync.dma_start(t[:], dd[:])
                lnw.append(t)
            ln1w, ln1b, ln2w, ln2b = lnw
            q1m = fcon.tile([128, SB], F32, tag="q1m")
            nc.sync.dma_start(q1m[:], q1m_d[:])
            qp = fcon.tile([128, SB], F32, tag="qp")
            nc.sync.dma_start(qp[:], qp_d[:])

            def layer_norm(x_ap, w_ap, b_ap, out_ap):
                G = D // 512
                st6 = smalls.tile([128, 6 * G], F32, tag="st6")
                for gg in range(G):
                    nc.vector.bn_stats(st6[:, 6 * gg:6 * gg + 6],
                                       x_ap[:, 512 * gg:512 * gg + 512])
                mv = smalls.tile([128, 2], F32, tag="mv")
                nc.vector.bn_aggr(mv[:], st6[:])
                ve = smalls.tile([128, 1], F32, tag="ve")
                nc.vector.tensor_scalar_add(ve[:], mv[:, 1:2], EPS)
                sd = smalls.tile([128, 1], F32, tag="sd")
                nc.scalar.sqrt(sd[:], ve[:])
                rs_ = smalls.tile([128, 1], F32, tag="rs")
                nc.vector.reciprocal(rs_[:], sd[:])
                xc = stg.tile([128, D], F32, tag="stg")
                nc.vector.tensor_scalar_sub(xc[:], x_ap, mv[:, 0:1])
                nc.vector.scalar_tensor_tensor(out_ap, xc[:], rs_[:], w_ap,
                                               ALU.mult, ALU.mult)
                nc.vector.tensor_add(out_ap, out_ap, b_ap)

            for half in range(HALVES):
                rsrc = rs_outs[half]
                # RS-gated loads go on the gpsimd queue (behind the RS
                # trigger in FIFO order) so a not-yet-ready load can never
                # block the sync queue's ready work; wait_until pins the
                # scheduler from hoisting them earlier.
                ldq = nc.sync if half == 0 else nc.gpsimd
                vtr = fcon.tile([1, D], F16, tag="vtr")
                ldq.dma_start(vtr[:], rsrc[512:513, :])
                rsbs = []
                for tl in range(TBH):
                    row = tl * 128
                    rsb = rsp.tile([128, D], F16, tag="rsb")
                    ldq.dma_start(rsb[:], rsrc[row:row + 128, :])
                    rsbs.append(rsb)
                # broadcast this half's vtot row across partitions
                vtrb = fcon.tile([1, D], BF16, tag="vtrb")
                nc.scalar.copy(vtrb[:], vtr[:])
                vtb = fcon.tile([128, D], F32, tag="vtb")
                for ec in range(EC):
                    bps = pmed.tile([128, 512], F32, tag="med")
                    nc.tensor.matmul(bps[:], ones_bf[0:1, 0:128],
                                     vtrb[0:1, ec * 512:ec * 512 + 512],
                                     start=True, stop=True)
                    nc.scalar.copy(vtb[:, ec * 512:ec * 512 + 512], bps[:])

                x1s = []
                # ---- x0 = blend(attn) + residual; x1 = LN1(x0) ----
                for tl in range(TBH):
                    sblk = half * TBH + tl
                    xsb = stg.tile([128, D], F32, tag="stg")
                    nc.sync.dma_start(xsb[:], xs_d[sblk * 128:sblk * 128 + 128, :])
                    t0 = stg.tile([128, D], F32, tag="stg")
                    nc.vector.scalar_tensor_tensor(
                        t0[:], rsbs[tl][:], q1m[:, sblk:sblk + 1], xsb[:],
                        ALU.mult, ALU.add)
                    t1 = stg.tile([128, D], F32, tag="stg")
                    nc.vector.scalar_tensor_tensor(
                        t1[:], bvs[:], q1m[:, sblk:sblk + 1], t0[:],
                        ALU.mult, ALU.add)
                    x0 = stg.tile([128, D], F32, tag="stg")
                    nc.vector.scalar_tensor_tensor(
                        x0[:], vtb[:], qp[:, sblk:sblk + 1], t1[:],
                        ALU.mult, ALU.add)
                    x1 = x1p.tile([128, D], F32, tag="x1")
                    layer_norm(x0[:], ln1w[:], ln1b[:], x1[:])
                    x1s.append(x1)

                # ---- x1^T (bf16) ----
                x1t = x1tp.tile([128, DC * TPH], BF16, tag="x1t")
                for tl in range(TBH):
                    for c in range(DC):
                        tp = pmed.tile([128, 128], F32, tag="med")
                        nc.tensor.transpose(
                            tp[:], x1s[tl][:, c * 128:c * 128 + 128], ident[:])
                        nc.scalar.copy(
                            x1t[:, c * TPH + tl * 128:c * TPH + tl * 128 + 128],
                            tp[:])

                # ---- hT = relu(W1^T x1^T + b1) (bf16) ----
                hts = htp.tile([128, FB * TPH], BF16, tag="hts")
                HCH = TPH // 256
                for hc in range(HCH):
                    for fb in range(FB):
                        w1s = w1p.tile([128, D], BF16, tag="w1s")
                        nc.sync.dma_start(w1s[:], w1_d[fb])
                        hps = pmed.tile([128, 256], F32, tag="med")
                        for c in range(DC):
                            nc.tensor.matmul(
                                hps[:], w1s[:, c * 128:c * 128 + 128],
                                x1t[:, c * TPH + hc * 256:
                                    c * TPH + hc * 256 + 256],
                                start=(c == 0), stop=(c == DC - 1))
                        nc.scalar.activation(
                            hts[:, fb * TPH + hc * 256:
                                fb * TPH + hc * 256 + 256],
                            hps[:], AF.Relu,
                            bias=b1t[:, fb:fb + 1], scale=1.0)

                # ---- y = hT.T @ W2 accumulated over fb groups ----
                ys_prev = [None] * TBH
                NG = FB // GF
                w2cache = {}
                for g in range(NG):
                    for tl in range(TBH):
                        yps = pbig.tile([128, D], F32, tag="big")
                        for j in range(GF):
                            fb = g * GF + j
                            if tl == 0:
                                w2s = w2p.tile([128, D], BF16, tag="w2s")
                                nc.sync.dma_start(
                                    w2s[:], w2_d[fb * 128:fb * 128 + 128, :])
                                w2cache[fb] = w2s
                            w2s = w2cache[fb]
                            for ec in range(EC):
                                nc.tensor.matmul(
                                    yps[:, ec * 512:ec * 512 + 512],
                                    hts[:, fb * TPH + tl * 128:
                                        fb * TPH + tl * 128 + 128],
                                    w2s[:, ec * 512:ec * 512 + 512],
                                    start=(j == 0), stop=(j == GF - 1))
                        if g < NG - 1:
                            ysn = ysp.tile([128, D], F32, tag="ysb")
                            if g == 0:
                                nc.scalar.copy(ysn[:], yps[:])
                            else:
                                nc.vector.scalar_tensor_tensor(
                                    ysn[:], yps[:], 1.0, ys_prev[tl][:],
                                    ALU.mult, ALU.add)
                            ys_prev[tl] = ysn
                        else:
                            # x2 = x1 + (y_partial + yps) + b2; out = LN2(x2)
                            x2 = stg.tile([128, D], F32, tag="stg")
                            nc.vector.scalar_tensor_tensor(
                                x2[:], yps[:], 1.0, ys_prev[tl][:],
                                ALU.mult, ALU.add)
                            nc.vector.tensor_add(x2[:], x2[:], x1s[tl][:])
                            nc.vector.tensor_add(x2[:], x2[:], b2b[:])
                            ot = stg.tile([128, D], F32, tag="stg")
                            layer_norm(x2[:], ln2w[:], ln2b[:], ot[:])
                            row = (half * TBH + tl) * 128
                            nc.gpsimd.dma_start(out_d[row:row + 128, :], ot[:])

    nc.compile()
    return nc


def _oext_rows(o_exts, spp, b, ic, s, S):
    """Rows [128] of the o_ext tile for query block (b, ic, s).

    spp = slices per RS phase; each 513-row slice = 512 tokens + 1 vmean row.
    """
    grow = b * S + ic * 256 + s * 128
    sl = grow // 512
    row0 = (sl % spp) * 513 + grow % 512
    return o_exts[sl // spp][row0:row0 + 128, :]


# ------------------------- host side -------------------------

_NC_CACHE = {}


def _get_nc(cfg_key):
    if cfg_key not in _NC_CACHE:
        _NC_CACHE[cfg_key] = build_nc(**CFG_MAIN)
    return _NC_CACHE[cfg_key]


def make_in_maps(inputs, B, S, D, FF, ncores, HPC):
    """Build the per-core input dicts from the full (unsharded) inputs."""
    TB = S // 128
    DC = D // 128
    shard = B * S // ncores
    SB = shard // 128
    FB = FF // 128
    H = ncores * HPC
    NS = B * S // 512
    HALVES = NS // ncores
    bf = ml_dtypes.bfloat16
    f8 = ml_dtypes.float8_e4m3

    x = np.ascontiguousarray(
        np.asarray(inputs["input"], dtype=np.float32).reshape(B * S, D))
    xT = np.ascontiguousarray(x.T)
    xTb = xT.astype(bf)
    xT8 = np.clip(xT, -240.0, 240.0).astype(f8)
    pad = np.asarray(inputs["padding_mask"], dtype=bool)
    Wq = np.asarray(inputs["Wq"], dtype=np.float32)
    Wk = np.asarray(inputs["Wk"], dtype=np.float32)
    Wv = np.asarray(inputs["Wv"], dtype=np.float32)
    bq = np.asarray(inputs["bq"], dtype=np.float32)
    bk = np.asarray(inputs["bk"], dtype=np.float32)
    bvv = np.asarray(inputs["bv"], dtype=np.float32)

    padb = np.where(pad, np.float32(NEG), np.float32(0.0)) - np.float32(ESHIFT)
    padb = np.ascontiguousarray(
        padb.reshape(B, TB, 128).transpose(0, 2, 1))

    cmask = np.zeros((128, 128), dtype=np.float32)
    cmask[np.tril_indices(128, -1)] = NEG

    w1 = np.asarray(inputs["ff1_w"], dtype=np.float32)
    w1s = np.ascontiguousarray(
        w1.reshape(D // 128, 128, FB, 128).transpose(2, 1, 0, 3)
        .reshape(FB, 128, D)).astype(bf)
    w2 = np.asarray(inputs["ff2_w"], dtype=np.float32).astype(bf)
    b1 = np.asarray(inputs["ff1_b"], dtype=np.float32)
    b1t = np.ascontiguousarray(b1.reshape(FB, 128).T)
    b2b = np.ascontiguousarray(
        np.broadcast_to(np.asarray(inputs["ff2_b"], np.float32), (128, D)))
    bvs = np.ascontiguousarray(
        np.broadcast_to(bvv.sum(axis=0), (128, D)).astype(np.float32))

    def bc(name):
        return np.ascontiguousarray(np.broadcast_to(
            np.asarray(inputs[name], np.float32), (128, D)))

    ident = np.eye(128, dtype=np.float32)
    padflat = pad.reshape(B * S)

    in_maps = []
    for c in range(ncores):
        h0 = c * HPC
        # wqk[p, w, c, m]: m = (head0 kd 0..63 | head1 kd 0..63)
        wqk = np.empty((128, 2, DC, 128), dtype=np.float32)
        for w, W in ((0, Wq), (1, Wk)):
            for hh in range(HPC):
                Wr = W[h0 + hh].reshape(DC, 128, 64)  # [c, p, kd]
                wqk[:, w, :, hh * 64:(hh + 1) * 64] = Wr.transpose(1, 0, 2)
        wqk = np.ascontiguousarray(wqk.reshape(128, 2 * DC * 128)).astype(bf)
        bqk = np.empty((1, 2 * 128), dtype=np.float32)
        for w, bb in ((0, bq), (1, bk)):
            for hh in range(HPC):
                bqk[0, w * 128 + hh * 64:w * 128 + (hh + 1) * 64] = bb[h0 + hh]
        bqk = bqk.astype(bf)
        # wv8[h, p, c*D + e] = 16*Wv[h, c*128+p, e]
        wv8 = np.ascontiguousarray(
            (Wv[h0:h0 + HPC] * np.float32(WVS))
            .reshape(HPC, DC, 128, D).transpose(0, 2, 1, 3)
            .reshape(HPC, 128, DC * D))
        wv8 = np.clip(wv8, -240.0, 240.0).astype(f8)

        # core c owns slice h*(NS//HALVES)+c for each RS phase h
        tok_idx = np.concatenate([
            np.arange(512 * (h * (NS // HALVES) + c),
                      512 * (h * (NS // HALVES) + c) + 512)
            for h in range(HALVES)])
        prow = padflat[tok_idx].reshape(SB, 128).T
        prow = prow.astype(np.float32)
        m = {
            "xtb": xTb,
            "xt8": xT8,
            "xs": np.ascontiguousarray(x[tok_idx]),
            "wqk": wqk,
            "bqk": bqk,
            "wv8": wv8,
            "bv": np.ascontiguousarray(
                bvv[h0:h0 + HPC].reshape(1, -1)).astype(bf),
            "bvs": bvs,
            "padb": padb,
            "q1m": np.ascontiguousarray((1.0 - prow) / H),
            "qp": np.ascontiguousarray(prow / H),
            "cmask": cmask,
            "ident": ident,
            "w1s": w1s,
            "w2": w2,
            "b1t": b1t,
            "b2b": b2b,
            "ln1w": bc("ln1_w"),
            "ln1b": bc("ln1_b"),
            "ln2w": bc("ln2_w"),
            "ln2b": bc("ln2_b"),
        }
        in_maps.append(m)
    return in_maps


def kernel(**inputs):
    from concourse.bass_utils import run_bass_kernel_spmd
    cfg = CFG_MAIN
    B, S, D = cfg["B"], cfg["S"], cfg["D"]
    ncores = cfg["ncores"]
    shard = B * S // ncores
    nc = _get_nc("main")
    in_maps = make_in_maps(inputs, **cfg)
    res = run_bass_kernel_spmd(nc, in_maps, list(range(ncores)))
    NS = B * S // 512
    HALVES = NS // ncores
    out = np.empty((B * S, D), dtype=np.float32)
    for c in range(ncores):
        r_ = np.asarray(res.results[c]["out"])
        for h in range(HALVES):
            sl = h * (NS // HALVES) + c
            out[512 * sl:512 * sl + 512] = r_[512 * h:512 * h + 512]
    return out.reshape(B, S, D).astype(np.float32)


# revision 13
# speedup vs baseline: 1.1424x; 1.1424x over previous
"""Trainium2 Bass kernel for nn_DecoderLayer (B=4,S=2048,D=1024,H=16,FF=4096).

Sharding: 16 heads / 8 cores = 2 heads per core (tensor/head parallel) for
attention; ReduceScatter (fp16) of the head-summed attention output;
token-parallel LN+FFN on each core's 1/8 token shard; host concatenates.

Numerics: host supplies X^T in bf16 and fp8(e4m3); Q/K projections bf16
(2-head-packed stationary, bias via a K=1 matmul into the same PSUM group);
V projection and P@V run fp8 DoubleRow (2x contraction per instruction) with
exp shifted by -3.5 so softmax numerators fit fp8 range; Wv is scaled x16 on
host so its values sit in fp8 normal range (descaled at V eviction);
softmax/LN in fp32; FFN matmuls bf16; head outputs accumulated in DRAM fp16.
"""
import numpy as np
import ml_dtypes
from contextlib import ExitStack

import concourse.bass as bass
import concourse.tile as tile
from concourse import bacc, mybir

dt = mybir.dt
F32 = dt.float32
F16 = dt.float16
BF16 = dt.bfloat16
FP8 = dt.float8e4
AF = mybir.ActivationFunctionType
ALU = mybir.AluOpType
AX = mybir.AxisListType
DR = mybir.MatmulPerfMode.DoubleRow

KD = 64
EPS = 1e-5
NEG = -30000.0
ESHIFT = 3.5          # exp(score - ESHIFT) keeps numerators in fp8 range
WVS = 16.0            # host scales Wv by x16 into fp8 normal range

CFG_MAIN = dict(B=4, S=2048, D=1024, FF=4096, ncores=8, HPC=2)


def build_nc(B, S, D, FF, ncores, HPC):
    DC = D // 128          # d chunks
    CP = DC // 2           # d-chunk pairs (DoubleRow)
    TB = S // 128          # t blocks per batch
    IC = S // 256          # i chunks (256 queries) per batch
    EC = D // 512          # 512-wide e chunks
    TOKC = S // 512
    shard = B * S // ncores
    SB = shard // 128
    FB = FF // 128
    HALVES = max(1, shard // 512)
    TPH = shard // HALVES  # tokens per FFN half
    TBH = TPH // 128
    GF = 4                 # fb group size for y accumulation
    SPB = S // 512         # 512-token slices per batch
    NS = B * S // 512      # total slices
    assert NS % ncores == 0 and HALVES == NS // ncores

    nc = bacc.Bacc("TRN2", target_bir_lowering=False, debug=False,
                   enable_asserts=False, num_devices=ncores)

    # ---- DRAM I/O ----
    xt_d = nc.dram_tensor("xtb", [D, B * S], BF16, kind="ExternalInput").ap()
    xt8_d = nc.dram_tensor("xt8", [D, B * S], FP8, kind="ExternalInput").ap()
    xs_d = nc.dram_tensor("xs", [shard, D], F32, kind="ExternalInput").ap()
    wqk_d = nc.dram_tensor("wqk", [128, 2 * DC * 128], BF16,
                           kind="ExternalInput").ap()
    bqk_d = nc.dram_tensor("bqk", [1, 2 * 128], BF16, kind="ExternalInput").ap()
    wv8_d = nc.dram_tensor("wv8", [HPC, 128, DC * D], FP8,
                           kind="ExternalInput").ap()
    bv_d = nc.dram_tensor("bv", [1, HPC * D], BF16, kind="ExternalInput").ap()
    bvs_d = nc.dram_tensor("bvs", [128, D], F32, kind="ExternalInput").ap()
    padb_d = nc.dram_tensor("padb", [B, 128, TB], F32, kind="ExternalInput").ap()
    q1m_d = nc.dram_tensor("q1m", [128, SB], F32, kind="ExternalInput").ap()
    qp_d = nc.dram_tensor("qp", [128, SB], F32, kind="ExternalInput").ap()
    cm_d = nc.dram_tensor("cmask", [128, 128], F32, kind="ExternalInput").ap()
    id_d = nc.dram_tensor("ident", [128, 128], F32, kind="ExternalInput").ap()
    w1_d = nc.dram_tensor("w1s", [FB, 128, D], BF16, kind="ExternalInput").ap()
    w2_d = nc.dram_tensor("w2", [FF, D], BF16, kind="ExternalInput").ap()
    b1t_d = nc.dram_tensor("b1t", [128, FB], F32, kind="ExternalInput").ap()
    b2b_d = nc.dram_tensor("b2b", [128, D], F32, kind="ExternalInput").ap()
    ln1w_d = nc.dram_tensor("ln1w", [128, D], F32, kind="ExternalInput").ap()
    ln1b_d = nc.dram_tensor("ln1b", [128, D], F32, kind="ExternalInput").ap()
    ln2w_d = nc.dram_tensor("ln2w", [128, D], F32, kind="ExternalInput").ap()
    ln2b_d = nc.dram_tensor("ln2b", [128, D], F32, kind="ExternalInput").ap()
    out_d = nc.dram_tensor("out", [shard, D], F32, kind="ExternalOutput").ap()

    with tile.TileContext(nc) as tc, ExitStack() as ctx0:
        pbig = ctx0.enter_context(tc.tile_pool(name="pbig", bufs=2, space="PSUM"))
        pmed = ctx0.enter_context(tc.tile_pool(name="pmed", bufs=2, space="PSUM"))
        pdn = ctx0.enter_context(tc.tile_pool(name="pdn", bufs=2, space="PSUM"))
        dramp = ctx0.enter_context(tc.tile_pool(name="dram", bufs=1, space="DRAM"))
        consts = ctx0.enter_context(tc.tile_pool(name="const", bufs=1))
        smalls = ctx0.enter_context(tc.tile_pool(name="smalls", bufs=6))

        o_exts = [dramp.tile([(NS // HALVES) * 513, D], F16,
                             name=f"oext{rp}")
                  for rp in range(HALVES)]
        rs_outs = [dramp.tile([513, D], F16, name=f"rsout{rp}")
                   for rp in range(HALVES)]

        # ---- constants ----
        ident = consts.tile([128, 128], F32, tag="ident")
        nc.sync.dma_start(ident[:], id_d[:])
        cmask = consts.tile([128, 128], F32, tag="cmask")
        nc.sync.dma_start(cmask[:], cm_d[:])
        ones_bf = consts.tile([1, 512], BF16, tag="ones_bf")
        nc.vector.memset(ones_bf[:], 1.0)
        ones8 = consts.tile([128, 32], FP8, tag="ones8")
        nc.vector.memset(ones8[:], 1.0)
        ones8_3 = ones8[:].rearrange("p (j u) -> p j u", u=16)[:, :, 0:1]
        # =================== attention phase ===================
        with ExitStack() as actx:
            xtp = actx.enter_context(tc.tile_pool(name="xt", bufs=2))
            xt8p = actx.enter_context(tc.tile_pool(name="xt8", bufs=2))
            wvp = actx.enter_context(tc.tile_pool(name="wv", bufs=2))
            vp = actx.enter_context(tc.tile_pool(name="v", bufs=2))
            vrp = actx.enter_context(tc.tile_pool(name="vr", bufs=2))
            qkp = actx.enter_context(tc.tile_pool(name="qkt", bufs=2))
            ep = actx.enter_context(tc.tile_pool(name="e", bufs=6))
            osbp = actx.enter_context(tc.tile_pool(name="osb", bufs=4))
            padp = actx.enter_context(tc.tile_pool(name="pad", bufs=3))
            aconp = actx.enter_context(tc.tile_pool(name="acon", bufs=1))

            wqk = aconp.tile([128, 2 * DC * 128], BF16, tag="wqk")
            nc.sync.dma_start(wqk[:], wqk_d[:])
            wqk4 = wqk[:].rearrange("p (w c m) -> p w c m", w=2, c=DC)
            bqk = aconp.tile([1, 2 * 128], BF16, tag="bqk")
            nc.sync.dma_start(bqk[:], bqk_d[:])
            bv = aconp.tile([1, HPC * D], BF16, tag="bv")
            nc.sync.dma_start(bv[:], bv_d[:])

            for b in range(B):
                # ---- load X^T tiles (bf16 for QK, fp8 for V) ----
                xt = xtp.tile([128, DC * S], BF16, tag="xt")
                xt8 = xt8p.tile([128, DC * S], FP8, tag="xt8")
                for c in range(DC):
                    nc.sync.dma_start(
                        xt[:, c * S:(c + 1) * S],
                        xt_d[c * 128:(c + 1) * 128, b * S:(b + 1) * S])
                    nc.sync.dma_start(
                        xt8[:, c * S:(c + 1) * S],
                        xt8_d[c * 128:(c + 1) * 128, b * S:(b + 1) * S])
                xt8_3 = xt8[:].rearrange("p (c t) -> p c t", c=DC)
                # column sums of X (for fully-padded-row vmean)
                xsum = smalls.tile([128, DC], F32, tag="xsum")
                for c in range(DC):
                    nc.vector.tensor_reduce(
                        xsum[:, c:c + 1], xt[:, c * S:(c + 1) * S],
                        AX.X, ALU.add)
                xsum8 = smalls.tile([128, DC], FP8, tag="xsum8")
                nc.vector.tensor_copy(out=xsum8[:], in_=xsum[:])

                padt = padp.tile([128, TB], F32, tag="pad")
                nc.sync.dma_start(padt[:], padb_d[b])

                # ---- Q/K projection, both heads packed (M=128) ----
                qkt2 = qkp.tile([128, 2 * S], BF16, tag="qkt")
                for w in range(2):
                    for c4 in range(TOKC):
                        ps = pmed.tile([128, 512], F32, tag="med")
                        for c in range(DC):
                            nc.tensor.matmul(
                                ps[:], wqk4[:, w, c, :],
                                xt[:, c * S + c4 * 512:c * S + c4 * 512 + 512],
                                start=(c == 0), stop=False)
                        nc.tensor.matmul(
                            ps[:], bqk[0:1, w * 128:w * 128 + 128],
                            ones_bf[0:1, :], start=False, stop=True)
                        off = w * S + c4 * 512
                        nc.scalar.activation(
                            qkt2[:, off:off + 512], ps[:], AF.Copy,
                            scale=(0.125 if w == 0 else 1.0))

                # ---- V projection per head (fp8 DoubleRow) + vmean ----
                v8s = []
                for h in range(HPC):
                    wv8 = wvp.tile([128, DC * D], FP8, tag="wv")
                    nc.sync.dma_start(wv8[:], wv8_d[h])
                    wv8_3 = wv8[:].rearrange("p (c e) -> p c e", c=DC)
                    v8 = vp.tile([128, TB * D], FP8, tag="v", name=f"v8_{h}")
                    for tb in range(TB):
                        vps = pbig.tile([128, D], F32, tag="big")
                        for cc in range(CP):
                            for ec in range(EC):
                                nc.tensor.matmul(
                                    vps[:, ec * 512:ec * 512 + 512],
                                    xt8_3[:, 2 * cc:2 * cc + 2,
                                          tb * 128:tb * 128 + 128],
                                    wv8_3[:, 2 * cc:2 * cc + 2,
                                          ec * 512:ec * 512 + 512],
                                    start=(cc == 0), stop=(cc == CP - 1),
                                    perf_mode=DR)
                        nc.scalar.activation(v8[:, tb * D:tb * D + D], vps[:],
                                             AF.Copy, scale=1.0 / WVS)

                    # vmean (for fully-padded rows), via xsum
                    vmps = [pmed.tile([1, 512], F32, tag="med",
                                      name=f"vmps{ec}")
                            for ec in range(EC)]
                    for ec in range(EC):
                        for c in range(DC):
                            nc.tensor.matmul(
                                vmps[ec][:], xsum8[:, c:c + 1],
                                wv8[:, c * D + ec * 512:c * D + ec * 512 + 512],
                                start=(c == 0), stop=(c == DC - 1))
                    vrow = vrp.tile([1, D], F16, tag="vrow")
                    for ec in range(EC):
                        nc.vector.scalar_tensor_tensor(
                            vrow[0:1, ec * 512:ec * 512 + 512], vmps[ec][:],
                            1.0 / (WVS * S),
                            bv[0:1, h * D + ec * 512:h * D + ec * 512 + 512],
                            ALU.mult, ALU.add)
                    spp = NS // HALVES
                    for k in range(SPB):
                        sl = b * SPB + k
                        vr = (sl % spp) * 513 + 512
                        nc.gpsimd.dma_start(
                            o_exts[sl // spp][vr:vr + 1, :], vrow[0:1, :],
                            accum_op=(ALU.bypass if h == 0 else ALU.add))
                    v8s.append(v8)

                # ---- attention per head (scores bf16, P@V fp8 DoubleRow) ----
                for h in range(HPC):
                    hb = h * 64
                    v8_3 = v8s[h][:].rearrange("p (t e) -> p t e", t=TB)
                    for ic in range(IC):
                        ops = [pbig.tile([128, D], F32, tag="big",
                                         name=f"ops{s}")
                               for s in range(2)]
                        dnp = [pdn.tile([128, 1], F32, tag="dn",
                                        name=f"dnp{s}")
                               for s in range(2)]
                        npair = ic + 1

                        def mk_e(k):
                            e8 = ep.tile([128, 512], FP8, tag="e",
                                         name=f"e{k}")
                            for j in range(2):
                                tb = 2 * k + j
                                st = pmed.tile([128, 256], F32, tag="med",
                                               name=f"st{tb}")
                                nc.tensor.matmul(
                                    st[:],
                                    qkt2[hb:hb + 64,
                                         S + tb * 128:S + tb * 128 + 128],
                                    qkt2[hb:hb + 64,
                                         ic * 256:ic * 256 + 256],
                                    start=True, stop=True)
                                if tb >= 2 * ic:
                                    off = (tb - 2 * ic) * 128
                                    nc.vector.tensor_add(
                                        st[:, off:off + 128],
                                        st[:, off:off + 128], cmask[:])
                                nc.scalar.activation(
                                    e8[:, j * 256:j * 256 + 256], st[:],
                                    AF.Exp, bias=padt[:, tb:tb + 1],
                                    scale=1.0)
                            if k == ic:
                                # pair-last slab, s=0 columns: fully masked
                                nc.vector.memset(e8[:, 256:384], 0.0)
                            return e8

                        epipe = {0: mk_e(0)}
                        if npair > 1:
                            epipe[1] = mk_e(1)
                        for k in range(npair):
                            e8 = epipe.pop(k)
                            if k + 2 < npair:
                                epipe[k + 2] = mk_e(k + 2)
                            e8_3 = e8[:].rearrange("p (j q) -> p j q", j=2)
                            for s in range(2):
                                el = e8_3[:, :, s * 128:s * 128 + 128]
                                for ec in range(EC):
                                    nc.tensor.matmul(
                                        ops[s][:, ec * 512:ec * 512 + 512],
                                        el,
                                        v8_3[:, 2 * k:2 * k + 2,
                                             ec * 512:ec * 512 + 512],
                                        start=(k == 0), stop=(k == npair - 1),
                                        perf_mode=DR)
                                nc.tensor.matmul(
                                    dnp[s][:], el, ones8_3,
                                    start=(k == 0), stop=(k == npair - 1),
                                    perf_mode=DR)
                        for s in range(2):
                            dsb = smalls.tile([128, 1], F32, tag="dsb")
                            nc.vector.tensor_scalar_add(dsb[:], dnp[s][:],
                                                        1e-30)
                            rd = smalls.tile([128, 1], F32, tag="rd")
                            nc.vector.reciprocal(rd[:], dsb[:])
                            osb = osbp.tile([128, D], F16, tag="osb")
                            nc.scalar.activation(osb[:], ops[s][:], AF.Copy,
                                                 scale=rd[:])
                            nc.gpsimd.dma_start(
                                _oext_rows(o_exts, NS // HALVES, b, ic, s, S),
                                osb[:],
                                accum_op=(ALU.bypass if h == 0 else ALU.add))

                # trigger the RS phase whose batches just completed; the
                # collective runs on the CC cores and overlaps later batches
                bpp = B // HALVES
                if (b + 1) % bpp == 0:
                    rp = (b + 1) // bpp - 1
                    nc.gpsimd.collective_compute(
                        "ReduceScatter", ALU.add,
                        replica_groups=[list(range(ncores))],
                        ins=[o_exts[rp].opt()], outs=[rs_outs[rp].opt()])

        # =================== LN + FFN on the token shard ===================
        with ExitStack() as fctx:
            fcon = fctx.enter_context(tc.tile_pool(name="fcon", bufs=1))
            stg = fctx.enter_context(tc.tile_pool(name="stg", bufs=6))
            x1p = fctx.enter_context(tc.tile_pool(name="x1", bufs=TBH + 2))
            x1tp = fctx.enter_context(tc.tile_pool(name="x1t", bufs=2))
            htp = fctx.enter_context(tc.tile_pool(name="hts", bufs=1))
            w1p = fctx.enter_context(tc.tile_pool(name="w1s", bufs=6))
            w2p = fctx.enter_context(tc.tile_pool(name="w2s", bufs=GF + 2))
            ysp = fctx.enter_context(tc.tile_pool(name="ysb", bufs=2 * TBH))
            rsp = fctx.enter_context(tc.tile_pool(name="rsb", bufs=TBH + 2))

            b1t = fcon.tile([128, FB], F32, tag="b1t")
            nc.sync.dma_start(b1t[:], b1t_d[:])
            b2b = fcon.tile([128, D], F32, tag="b2b")
            nc.sync.dma_start(b2b[:], b2b_d[:])
            bvs = fcon.tile([128, D], F32, tag="bvs")
            nc.sync.dma_start(bvs[:], bvs_d[:])
            lnw = []
            for nm, dd in [("ln1w", ln1w_d), ("ln1b", ln1b_d),
                           ("ln2w", ln2w_d), ("ln2b", ln2b_d)]:
                t = fcon.tile([128, D], F32, tag=nm)
                nc.sync.dma_start(t[:], dd[:])
                lnw.append(t)
            ln1w, ln1b, ln2w, ln2b = lnw
            q1m = fcon.tile([128, SB], F32, tag="q1m")
            nc.sync.dma_start(q1m[:], q1m_d[:])
            qp = fcon.tile([128, SB], F32, tag="qp")
            nc.sync.dma_start(qp[:], qp_d[:])

            def layer_norm(x_ap, w_ap, b_ap, out_ap):
                G = D // 512
                st6 = smalls.tile([128, 6 * G], F32, tag="st6")
                for gg in range(G):
                    nc.vector.bn_stats(st6[:, 6 * gg:6 * gg + 6],
                                       x_ap[:, 512 * gg:512 * gg + 512])
                mv = smalls.tile([128, 2], F32, tag="mv")
                nc.vector.bn_aggr(mv[:], st6[:])
                ve = smalls.tile([128, 1], F32, tag="ve")
                nc.vector.tensor_scalar_add(ve[:], mv[:, 1:2], EPS)
                sd = smalls.tile([128, 1], F32, tag="sd")
                nc.scalar.sqrt(sd[:], ve[:])
                rs_ = smalls.tile([128, 1], F32, tag="rs")
                nc.vector.reciprocal(rs_[:], sd[:])
                xc = stg.tile([128, D], F32, tag="stg")
                nc.vector.tensor_scalar_sub(xc[:], x_ap, mv[:, 0:1])
                nc.vector.scalar_tensor_tensor(out_ap, xc[:], rs_[:], w_ap,
                                               ALU.mult, ALU.mult)
                nc.vector.tensor_add(out_ap, out_ap, b_ap)

            for half in range(HALVES):
                rsrc = rs_outs[half]
                vtr = fcon.tile([1, D], F16, tag="vtr")
                nc.gpsimd.dma_start(vtr[:], rsrc[512:513, :])
                rsbs = []
                for tl in range(TBH):
                    row = tl * 128
                    rsb = rsp.tile([128, D], F16, tag="rsb")
                    nc.gpsimd.dma_start(rsb[:], rsrc[row:row + 128, :])
                    rsbs.append(rsb)
                # broadcast this half's vtot row across partitions
                vtrb = fcon.tile([1, D], BF16, tag="vtrb")
                nc.scalar.copy(vtrb[:], vtr[:])
                vtb = fcon.tile([128, D], F32, tag="vtb")
                for ec in range(EC):
                    bps = pmed.tile([128, 512], F32, tag="med")
                    nc.tensor.matmul(bps[:], ones_bf[0:1, 0:128],
                                     vtrb[0:1, ec * 512:ec * 512 + 512],
                                     start=True, stop=True)
                    nc.scalar.copy(vtb[:, ec * 512:ec * 512 + 512], bps[:])

                x1s = []
                # ---- x0 = blend(attn) + residual; x1 = LN1(x0) ----
                for tl in range(TBH):
                    sblk = half * TBH + tl
                    xsb = stg.tile([128, D], F32, tag="stg")
                    nc.sync.dma_start(xsb[:], xs_d[sblk * 128:sblk * 128 + 128, :])
                    t0 = stg.tile([128, D], F32, tag="stg")
                    nc.vector.scalar_tensor_tensor(
                        t0[:], rsbs[tl][:], q1m[:, sblk:sblk + 1], xsb[:],
                        ALU.mult, ALU.add)
                    t1 = stg.tile([128, D], F32, tag="stg")
                    nc.vector.scalar_tensor_tensor(
                        t1[:], bvs[:], q1m[:, sblk:sblk + 1], t0[:],
                        ALU.mult, ALU.add)
                    x0 = stg.tile([128, D], F32, tag="stg")
                    nc.vector.scalar_tensor_tensor(
                        x0[:], vtb[:], qp[:, sblk:sblk + 1], t1[:],
                        ALU.mult, ALU.add)
                    x1 = x1p.tile([128, D], F32, tag="x1")
                    layer_norm(x0[:], ln1w[:], ln1b[:], x1[:])
                    x1s.append(x1)

                # ---- x1^T (bf16) ----
                x1t = x1tp.tile([128, DC * TPH], BF16, tag="x1t")
                for tl in range(TBH):
                    for c in range(DC):
                        tp = pmed.tile([128, 128], F32, tag="med")
                        nc.tensor.transpose(
                            tp[:], x1s[tl][:, c * 128:c * 128 + 128], ident[:])
                        nc.scalar.copy(
                            x1t[:, c * TPH + tl * 128:c * TPH + tl * 128 + 128],
                            tp[:])

                # ---- hT = relu(W1^T x1^T + b1) (bf16) ----
                hts = htp.tile([128, FB * TPH], BF16, tag="hts")
                for fb in range(FB):
                    w1s = w1p.tile([128, D], BF16, tag="w1s")
                    nc.sync.dma_start(w1s[:], w1_d[fb])
                    hps = pmed.tile([128, TPH], F32, tag="med")
                    for c in range(DC):
                        nc.tensor.matmul(hps[:], w1s[:, c * 128:c * 128 + 128],
                                         x1t[:, c * TPH:(c + 1) * TPH],
                                         start=(c == 0), stop=(c == DC - 1))
                    nc.scalar.activation(hts[:, fb * TPH:(fb + 1) * TPH],
                                         hps[:], AF.Relu,
                                         bias=b1t[:, fb:fb + 1], scale=1.0)

                # ---- y = hT.T @ W2 accumulated over fb groups ----
                ys_prev = [None] * TBH
                NG = FB // GF
                w2cache = {}
                for g in range(NG):
                    for tl in range(TBH):
                        yps = pbig.tile([128, D], F32, tag="big")
                        for j in range(GF):
                            fb = g * GF + j
                            if tl == 0:
                                w2s = w2p.tile([128, D], BF16, tag="w2s")
                                nc.sync.dma_start(
                                    w2s[:], w2_d[fb * 128:fb * 128 + 128, :])
                                w2cache[fb] = w2s
                            w2s = w2cache[fb]
                            for ec in range(EC):
                                nc.tensor.matmul(
                                    yps[:, ec * 512:ec * 512 + 512],
                                    hts[:, fb * TPH + tl * 128:
                                        fb * TPH + tl * 128 + 128],
                                    w2s[:, ec * 512:ec * 512 + 512],
                                    start=(j == 0), stop=(j == GF - 1))
                        ysn = ysp.tile([128, D], F32, tag="ysb")
                        if g == 0:
                            nc.scalar.copy(ysn[:], yps[:])
                        else:
                            nc.vector.scalar_tensor_tensor(
                                ysn[:], yps[:], 1.0, ys_prev[tl][:],
                                ALU.mult, ALU.add)
                        ys_prev[tl] = ysn

                # ---- x2 = x1 + y + b2; out = LN2(x2) ----
                for tl in range(TBH):
                    x2 = stg.tile([128, D], F32, tag="stg")
                    nc.vector.scalar_tensor_tensor(
                        x2[:], ys_prev[tl][:], 1.0, x1s[tl][:],
                        ALU.mult, ALU.add)
                    nc.vector.tensor_add(x2[:], x2[:], b2b[:])
                    ot = stg.tile([128, D], F32, tag="stg")
                    layer_norm(x2[:], ln2w[:], ln2b[:], ot[:])
                    row = (half * TBH + tl) * 128
                    nc.gpsimd.dma_start(out_d[row:row + 128, :], ot[:])

    nc.compile()
    return nc


def _oext_rows(o_exts, spp, b, ic, s, S):
    """Rows [128] of the o_ext tile for query block (b, ic, s).

    spp = slices per RS phase; each 513-row slice = 512 tokens + 1 vmean row.
    """
    grow = b * S + ic * 256 + s * 128
    sl = grow // 512
    row0 = (sl % spp) * 513 + grow % 512
    return o_exts[sl // spp][row0:row0 + 128, :]


# ------------------------- host side -------------------------

_NC_CACHE = {}


def _get_nc(cfg_key):
    if cfg_key not in _NC_CACHE:
        _NC_CACHE[cfg_key] = build_nc(**CFG_MAIN)
    return _NC_CACHE[cfg_key]


def make_in_maps(inputs, B, S, D, FF, ncores, HPC):
    """Build the per-core input dicts from the full (unsharded) inputs."""
    TB = S // 128
    DC = D // 128
    shard = B * S // ncores
    SB = shard // 128
    FB = FF // 128
    H = ncores * HPC
    NS = B * S // 512
    HALVES = NS // ncores
    bf = ml_dtypes.bfloat16
    f8 = ml_dtypes.float8_e4m3

    x = np.ascontiguousarray(
        np.asarray(inputs["input"], dtype=np.float32).reshape(B * S, D))
    xT = np.ascontiguousarray(x.T)
    xTb = xT.astype(bf)
    xT8 = np.clip(xT, -240.0, 240.0).astype(f8)
    pad = np.asarray(inputs["padding_mask"], dtype=bool)
    Wq = np.asarray(inputs["Wq"], dtype=np.float32)
    Wk = np.asarray(inputs["Wk"], dtype=np.float32)
    Wv = np.asarray(inputs["Wv"], dtype=np.float32)
    bq = np.asarray(inputs["bq"], dtype=np.float32)
    bk = np.asarray(inputs["bk"], dtype=np.float32)
    bvv = np.asarray(inputs["bv"], dtype=np.float32)

    padb = np.where(pad, np.float32(NEG), np.float32(0.0)) - np.float32(ESHIFT)
    padb = np.ascontiguousarray(
        padb.reshape(B, TB, 128).transpose(0, 2, 1))

    cmask = np.zeros((128, 128), dtype=np.float32)
    cmask[np.tril_indices(128, -1)] = NEG

    w1 = np.asarray(inputs["ff1_w"], dtype=np.float32)
    w1s = np.ascontiguousarray(
        w1.reshape(D // 128, 128, FB, 128).transpose(2, 1, 0, 3)
        .reshape(FB, 128, D)).astype(bf)
    w2 = np.asarray(inputs["ff2_w"], dtype=np.float32).astype(bf)
    b1 = np.asarray(inputs["ff1_b"], dtype=np.float32)
    b1t = np.ascontiguousarray(b1.reshape(FB, 128).T)
    b2b = np.ascontiguousarray(
        np.broadcast_to(np.asarray(inputs["ff2_b"], np.float32), (128, D)))
    bvs = np.ascontiguousarray(
        np.broadcast_to(bvv.sum(axis=0), (128, D)).astype(np.float32))

    def bc(name):
        return np.ascontiguousarray(np.broadcast_to(
            np.asarray(inputs[name], np.float32), (128, D)))

    ident = np.eye(128, dtype=np.float32)
    padflat = pad.reshape(B * S)

    in_maps = []
    for c in range(ncores):
        h0 = c * HPC
        # wqk[p, w, c, m]: m = (head0 kd 0..63 | head1 kd 0..63)
        wqk = np.empty((128, 2, DC, 128), dtype=np.float32)
        for w, W in ((0, Wq), (1, Wk)):
            for hh in range(HPC):
                Wr = W[h0 + hh].reshape(DC, 128, 64)  # [c, p, kd]
                wqk[:, w, :, hh * 64:(hh + 1) * 64] = Wr.transpose(1, 0, 2)
        wqk = np.ascontiguousarray(wqk.reshape(128, 2 * DC * 128)).astype(bf)
        bqk = np.empty((1, 2 * 128), dtype=np.float32)
        for w, bb in ((0, bq), (1, bk)):
            for hh in range(HPC):
                bqk[0, w * 128 + hh * 64:w * 128 + (hh + 1) * 64] = bb[h0 + hh]
        bqk = bqk.astype(bf)
        # wv8[h, p, c*D + e] = 16*Wv[h, c*128+p, e]
        wv8 = np.ascontiguousarray(
            (Wv[h0:h0 + HPC] * np.float32(WVS))
            .reshape(HPC, DC, 128, D).transpose(0, 2, 1, 3)
            .reshape(HPC, 128, DC * D))
        wv8 = np.clip(wv8, -240.0, 240.0).astype(f8)

        # core c owns slice h*(NS//HALVES)+c for each RS phase h
        tok_idx = np.concatenate([
            np.arange(512 * (h * (NS // HALVES) + c),
                      512 * (h * (NS // HALVES) + c) + 512)
            for h in range(HALVES)])
        prow = padflat[tok_idx].reshape(SB, 128).T
        prow = prow.astype(np.float32)
        m = {
            "xtb": xTb,
            "xt8": xT8,
            "xs": np.ascontiguousarray(x[tok_idx]),
            "wqk": wqk,
            "bqk": bqk,
            "wv8": wv8,
            "bv": np.ascontiguousarray(
                bvv[h0:h0 + HPC].reshape(1, -1)).astype(bf),
            "bvs": bvs,
            "padb": padb,
            "q1m": np.ascontiguousarray((1.0 - prow) / H),
            "qp": np.ascontiguousarray(prow / H),
            "cmask": cmask,
            "ident": ident,
            "w1s": w1s,
            "w2": w2,
            "b1t": b1t,
            "b2b": b2b,
            "ln1w": bc("ln1_w"),
            "ln1b": bc("ln1_b"),
            "ln2w": bc("ln2_w"),
            "ln2b": bc("ln2_b"),
        }
        in_maps.append(m)
    return in_maps


def kernel(**inputs):
    from concourse.bass_utils import run_bass_kernel_spmd
    cfg = CFG_MAIN
    B, S, D = cfg["B"], cfg["S"], cfg["D"]
    ncores = cfg["ncores"]
    shard = B * S // ncores
    nc = _get_nc("main")
    in_maps = make_in_maps(inputs, **cfg)
    res = run_bass_kernel_spmd(nc, in_maps, list(range(ncores)))
    NS = B * S // 512
    HALVES = NS // ncores
    out = np.empty((B * S, D), dtype=np.float32)
    for c in range(ncores):
        r_ = np.asarray(res.results[c]["out"])
        for h in range(HALVES):
            sl = h * (NS // HALVES) + c
            out[512 * sl:512 * sl + 512] = r_[512 * h:512 * h + 512]
    return out.reshape(B, S, D).astype(np.float32)


# revision 15
# speedup vs baseline: 1.3065x; 1.1436x over previous
"""Trainium2 Bass kernel for nn_DecoderLayer (B=4,S=2048,D=1024,H=16,FF=4096).

Sharding: 16 heads / 8 cores = 2 heads per core (tensor/head parallel) for
attention; ReduceScatter (fp16) of the head-summed attention output;
token-parallel LN+FFN on each core's 1/8 token shard; host concatenates.

Numerics: host supplies X^T in bf16 and fp8(e4m3); Q/K projections bf16
(2-head-packed stationary, bias via a K=1 matmul into the same PSUM group);
V projection and P@V run fp8 DoubleRow (2x contraction per instruction) with
exp shifted by -3.5 so softmax numerators fit fp8 range; Wv is scaled x16 on
host so its values sit in fp8 normal range (descaled at V eviction);
softmax/LN in fp32; FFN matmuls bf16; head outputs accumulated in DRAM fp16.
"""
import numpy as np
import ml_dtypes
from contextlib import ExitStack

import concourse.bass as bass
import concourse.tile as tile
from concourse import bacc, mybir

dt = mybir.dt
F32 = dt.float32
F16 = dt.float16
BF16 = dt.bfloat16
FP8 = dt.float8e4
AF = mybir.ActivationFunctionType
ALU = mybir.AluOpType
AX = mybir.AxisListType
DR = mybir.MatmulPerfMode.DoubleRow

KD = 64
EPS = 1e-5
NEG = -30000.0
ESHIFT = 3.5          # exp(score - ESHIFT) keeps numerators in fp8 range
WVS = 16.0            # host scales Wv by x16 into fp8 normal range

CFG_MAIN = dict(B=4, S=2048, D=1024, FF=4096, ncores=8, HPC=2)


def build_nc(B, S, D, FF, ncores, HPC):
    DC = D // 128          # d chunks
    CP = DC // 2           # d-chunk pairs (DoubleRow)
    TB = S // 128          # t blocks per batch
    IC = S // 256          # i chunks (256 queries) per batch
    EC = D // 512          # 512-wide e chunks
    TOKC = S // 512
    shard = B * S // ncores
    SB = shard // 128
    FB = FF // 128
    HALVES = max(1, shard // 512)
    TPH = shard // HALVES  # tokens per FFN half
    TBH = TPH // 128
    GF = 4                 # fb group size for y accumulation
    SPB = S // 512         # 512-token slices per batch
    NS = B * S // 512      # total slices
    assert NS % ncores == 0 and HALVES == NS // ncores

    nc = bacc.Bacc("TRN2", target_bir_lowering=False, debug=False,
                   enable_asserts=False, num_devices=ncores)

    # ---- DRAM I/O ----
    xt_d = nc.dram_tensor("xtb", [D, B * S], BF16, kind="ExternalInput").ap()
    xt8_d = nc.dram_tensor("xt8", [D, B * S], FP8, kind="ExternalInput").ap()
    xs_d = nc.dram_tensor("xs", [shard, D], F32, kind="ExternalInput").ap()
    wqk_d = nc.dram_tensor("wqk", [128, 2 * DC * 128], BF16,
                           kind="ExternalInput").ap()
    bqk_d = nc.dram_tensor("bqk", [1, 2 * 128], BF16, kind="ExternalInput").ap()
    wv8_d = nc.dram_tensor("wv8", [HPC, 128, DC * D], FP8,
                           kind="ExternalInput").ap()
    bv_d = nc.dram_tensor("bv", [1, HPC * D], BF16, kind="ExternalInput").ap()
    bvs_d = nc.dram_tensor("bvs", [128, D], F32, kind="ExternalInput").ap()
    padb_d = nc.dram_tensor("padb", [B, 128, TB], F32, kind="ExternalInput").ap()
    q1m_d = nc.dram_tensor("q1m", [128, SB], F32, kind="ExternalInput").ap()
    qp_d = nc.dram_tensor("qp", [128, SB], F32, kind="ExternalInput").ap()
    cm_d = nc.dram_tensor("cmask", [128, 128], F32, kind="ExternalInput").ap()
    id_d = nc.dram_tensor("ident", [128, 128], F32, kind="ExternalInput").ap()
    w1_d = nc.dram_tensor("w1s", [FB, 128, D], BF16, kind="ExternalInput").ap()
    w2_d = nc.dram_tensor("w2", [FF, D], BF16, kind="ExternalInput").ap()
    b1t_d = nc.dram_tensor("b1t", [128, FB], F32, kind="ExternalInput").ap()
    b2b_d = nc.dram_tensor("b2b", [128, D], F32, kind="ExternalInput").ap()
    ln1w_d = nc.dram_tensor("ln1w", [128, D], F32, kind="ExternalInput").ap()
    ln1b_d = nc.dram_tensor("ln1b", [128, D], F32, kind="ExternalInput").ap()
    ln2w_d = nc.dram_tensor("ln2w", [128, D], F32, kind="ExternalInput").ap()
    ln2b_d = nc.dram_tensor("ln2b", [128, D], F32, kind="ExternalInput").ap()
    out_d = nc.dram_tensor("out", [shard, D], F32, kind="ExternalOutput").ap()

    with tile.TileContext(nc) as tc, ExitStack() as ctx0:
        pbig = ctx0.enter_context(tc.tile_pool(name="pbig", bufs=2, space="PSUM"))
        pmed = ctx0.enter_context(tc.tile_pool(name="pmed", bufs=2, space="PSUM"))
        pdn = ctx0.enter_context(tc.tile_pool(name="pdn", bufs=1, space="PSUM"))
        dramp = ctx0.enter_context(tc.tile_pool(name="dram", bufs=1, space="DRAM"))
        consts = ctx0.enter_context(tc.tile_pool(name="const", bufs=1))
        smalls = ctx0.enter_context(tc.tile_pool(name="smalls", bufs=6))

        o_exts = [dramp.tile([(NS // HALVES) * 513, D], F16,
                             name=f"oext{rp}")
                  for rp in range(HALVES)]
        rs_outs = [dramp.tile([513, D], F16, name=f"rsout{rp}")
                   for rp in range(HALVES)]

        # ---- constants ----
        ident = consts.tile([128, 128], F32, tag="ident")
        nc.sync.dma_start(ident[:], id_d[:])
        cmask = consts.tile([128, 128], F32, tag="cmask")
        nc.sync.dma_start(cmask[:], cm_d[:])
        ones_bf = consts.tile([1, 512], BF16, tag="ones_bf")
        nc.vector.memset(ones_bf[:], 1.0)
        ones8 = consts.tile([128, 32], FP8, tag="ones8")
        nc.vector.memset(ones8[:], 1.0)
        ones8_3 = ones8[:].rearrange("p (j u) -> p j u", u=16)[:, :, 0:1]
        # =================== attention phase ===================
        with ExitStack() as actx:
            xtp = actx.enter_context(tc.tile_pool(name="xt", bufs=2))
            xt8p = actx.enter_context(tc.tile_pool(name="xt8", bufs=2))
            wvp = actx.enter_context(tc.tile_pool(name="wv", bufs=2))
            vp = actx.enter_context(tc.tile_pool(name="v", bufs=2))
            vrp = actx.enter_context(tc.tile_pool(name="vr", bufs=2))
            qkp = actx.enter_context(tc.tile_pool(name="qkt", bufs=2))
            ep = actx.enter_context(tc.tile_pool(name="e", bufs=6))
            osbp = actx.enter_context(tc.tile_pool(name="osb", bufs=4))
            padp = actx.enter_context(tc.tile_pool(name="pad", bufs=3))
            aconp = actx.enter_context(tc.tile_pool(name="acon", bufs=1))

            wqk = aconp.tile([128, 2 * DC * 128], BF16, tag="wqk")
            nc.sync.dma_start(wqk[:], wqk_d[:])
            wqk4 = wqk[:].rearrange("p (w c m) -> p w c m", w=2, c=DC)
            bqk = aconp.tile([1, 2 * 128], BF16, tag="bqk")
            nc.sync.dma_start(bqk[:], bqk_d[:])
            bv = aconp.tile([1, HPC * D], BF16, tag="bv")
            nc.sync.dma_start(bv[:], bv_d[:])

            for b in range(B):
                # ---- load X^T tiles (bf16 for QK, fp8 for V) ----
                xt = xtp.tile([128, DC * S], BF16, tag="xt")
                xt8 = xt8p.tile([128, DC * S], FP8, tag="xt8")
                for c in range(DC):
                    nc.sync.dma_start(
                        xt[:, c * S:(c + 1) * S],
                        xt_d[c * 128:(c + 1) * 128, b * S:(b + 1) * S])
                    nc.sync.dma_start(
                        xt8[:, c * S:(c + 1) * S],
                        xt8_d[c * 128:(c + 1) * 128, b * S:(b + 1) * S])
                xt8_3 = xt8[:].rearrange("p (c t) -> p c t", c=DC)
                # column sums of X (for fully-padded-row vmean)
                xsum = smalls.tile([128, DC], F32, tag="xsum")
                for c in range(DC):
                    nc.vector.tensor_reduce(
                        xsum[:, c:c + 1], xt[:, c * S:(c + 1) * S],
                        AX.X, ALU.add)
                xsum8 = smalls.tile([128, DC], FP8, tag="xsum8")
                nc.vector.tensor_copy(out=xsum8[:], in_=xsum[:])

                padt = padp.tile([128, TB], F32, tag="pad")
                nc.sync.dma_start(padt[:], padb_d[b])

                # ---- Q/K projection, both heads packed (M=128) ----
                qkt2 = qkp.tile([128, 2 * S], BF16, tag="qkt")
                for w in range(2):
                    for c4 in range(TOKC):
                        ps = pmed.tile([128, 512], F32, tag="med")
                        for c in range(DC):
                            nc.tensor.matmul(
                                ps[:], wqk4[:, w, c, :],
                                xt[:, c * S + c4 * 512:c * S + c4 * 512 + 512],
                                start=(c == 0), stop=False)
                        nc.tensor.matmul(
                            ps[:], bqk[0:1, w * 128:w * 128 + 128],
                            ones_bf[0:1, :], start=False, stop=True)
                        off = w * S + c4 * 512
                        nc.scalar.activation(
                            qkt2[:, off:off + 512], ps[:], AF.Copy,
                            scale=(0.125 if w == 0 else 1.0))

                # ---- V projection per head (fp8 DoubleRow) + vmean ----
                v8s = []
                for h in range(HPC):
                    wv8 = wvp.tile([128, DC * D], FP8, tag="wv")
                    nc.sync.dma_start(wv8[:], wv8_d[h])
                    wv8_3 = wv8[:].rearrange("p (c e) -> p c e", c=DC)
                    v8 = vp.tile([128, TB * D], FP8, tag="v", name=f"v8_{h}")
                    for tb in range(TB):
                        vps = pbig.tile([128, D], F32, tag="big")
                        for cc in range(CP):
                            for ec in range(EC):
                                nc.tensor.matmul(
                                    vps[:, ec * 512:ec * 512 + 512],
                                    xt8_3[:, 2 * cc:2 * cc + 2,
                                          tb * 128:tb * 128 + 128],
                                    wv8_3[:, 2 * cc:2 * cc + 2,
                                          ec * 512:ec * 512 + 512],
                                    start=(cc == 0), stop=(cc == CP - 1),
                                    perf_mode=DR)
                        nc.scalar.activation(v8[:, tb * D:tb * D + D], vps[:],
                                             AF.Copy, scale=1.0 / WVS)

                    # vmean (for fully-padded rows), via xsum
                    vmps = [pmed.tile([1, 512], F32, tag="med",
                                      name=f"vmps{ec}")
                            for ec in range(EC)]
                    for ec in range(EC):
                        for c in range(DC):
                            nc.tensor.matmul(
                                vmps[ec][:], xsum8[:, c:c + 1],
                                wv8[:, c * D + ec * 512:c * D + ec * 512 + 512],
                                start=(c == 0), stop=(c == DC - 1))
                    vrow = vrp.tile([1, D], F16, tag="vrow")
                    for ec in range(EC):
                        nc.vector.scalar_tensor_tensor(
                            vrow[0:1, ec * 512:ec * 512 + 512], vmps[ec][:],
                            1.0 / (WVS * S),
                            bv[0:1, h * D + ec * 512:h * D + ec * 512 + 512],
                            ALU.mult, ALU.add)
                    spp = NS // HALVES
                    for k in range(SPB):
                        sl = b * SPB + k
                        vr = (sl % spp) * 513 + 512
                        nc.gpsimd.dma_start(
                            o_exts[sl // spp][vr:vr + 1, :], vrow[0:1, :],
                            accum_op=(ALU.bypass if h == 0 else ALU.add))
                    v8s.append(v8)

                # ---- attention per head (scores bf16, P@V fp8 DoubleRow) ----
                for h in range(HPC):
                    hb = h * 64
                    v8_3 = v8s[h][:].rearrange("p (t e) -> p t e", t=TB)
                    for ic in range(IC):
                        ops = [pbig.tile([128, D], F32, tag="big",
                                         name=f"ops{s}")
                               for s in range(2)]
                        dnps = pdn.tile([1, 256], F32, tag="dn")
                        npair = ic + 1

                        def mk_e(k):
                            e8 = ep.tile([128, 512], FP8, tag="e",
                                         name=f"e{k}")
                            for j in range(2):
                                tb = 2 * k + j
                                st = pmed.tile([128, 256], F32, tag="med",
                                               name=f"st{tb}")
                                nc.tensor.matmul(
                                    st[:],
                                    qkt2[hb:hb + 64,
                                         S + tb * 128:S + tb * 128 + 128],
                                    qkt2[hb:hb + 64,
                                         ic * 256:ic * 256 + 256],
                                    start=True, stop=True)
                                if tb >= 2 * ic:
                                    off = (tb - 2 * ic) * 128
                                    nc.vector.tensor_add(
                                        st[:, off:off + 128],
                                        st[:, off:off + 128], cmask[:])
                                nc.scalar.activation(
                                    e8[:, j * 256:j * 256 + 256], st[:],
                                    AF.Exp, bias=padt[:, tb:tb + 1],
                                    scale=1.0)
                            if k == ic:
                                # pair-last slab, s=0 columns: fully masked
                                nc.vector.memset(e8[:, 256:384], 0.0)
                            return e8

                        epipe = {0: mk_e(0)}
                        if npair > 1:
                            epipe[1] = mk_e(1)
                        for k in range(npair):
                            e8 = epipe.pop(k)
                            if k + 2 < npair:
                                epipe[k + 2] = mk_e(k + 2)
                            e8_3 = e8[:].rearrange("p (j q) -> p j q", j=2)
                            # denominators for both s at once: ones is the
                            # stationary (2-col weight -> near-free LDW)
                            nc.tensor.matmul(
                                dnps[:], ones8_3, e8_3,
                                start=(k == 0), stop=(k == npair - 1),
                                perf_mode=DR)
                            for s in range(2):
                                el = e8_3[:, :, s * 128:s * 128 + 128]
                                for ec in range(EC):
                                    nc.tensor.matmul(
                                        ops[s][:, ec * 512:ec * 512 + 512],
                                        el,
                                        v8_3[:, 2 * k:2 * k + 2,
                                             ec * 512:ec * 512 + 512],
                                        start=(k == 0), stop=(k == npair - 1),
                                        perf_mode=DR)
                        dnb = smalls.tile([1, 256], BF16, tag="dnb")
                        nc.vector.tensor_copy(out=dnb[:], in_=dnps[:])
                        for s in range(2):
                            dtp = pdn.tile([128, 1], F32, tag="dnt")
                            nc.tensor.matmul(
                                dtp[:], dnb[0:1, s * 128:s * 128 + 128],
                                ones_bf[0:1, 0:1], start=True, stop=True)
                            dsb = smalls.tile([128, 1], F32, tag="dsb")
                            nc.vector.tensor_scalar_add(dsb[:], dtp[:],
                                                        1e-30)
                            rd = smalls.tile([128, 1], F32, tag="rd")
                            nc.vector.reciprocal(rd[:], dsb[:])
                            osb = osbp.tile([128, D], F16, tag="osb")
                            nc.scalar.activation(osb[:], ops[s][:], AF.Copy,
                                                 scale=rd[:])
                            nc.gpsimd.dma_start(
                                _oext_rows(o_exts, NS // HALVES, b, ic, s, S),
                                osb[:],
                                accum_op=(ALU.bypass if h == 0 else ALU.add))

                # trigger the RS phase whose batches just completed; the
                # collective runs on the CC cores and overlaps later batches
                bpp = B // HALVES
                if (b + 1) % bpp == 0:
                    rp = (b + 1) // bpp - 1
                    nc.gpsimd.collective_compute(
                        "ReduceScatter", ALU.add,
                        replica_groups=[list(range(ncores))],
                        ins=[o_exts[rp].opt()], outs=[rs_outs[rp].opt()])

        # =================== LN + FFN on the token shard ===================
        with ExitStack() as fctx:
            fcon = fctx.enter_context(tc.tile_pool(name="fcon", bufs=1))
            stg = fctx.enter_context(tc.tile_pool(name="stg", bufs=6))
            x1p = fctx.enter_context(tc.tile_pool(name="x1", bufs=TBH + 2))
            x1tp = fctx.enter_context(tc.tile_pool(name="x1t", bufs=2))
            htp = fctx.enter_context(tc.tile_pool(name="hts", bufs=1))
            w1p = fctx.enter_context(tc.tile_pool(name="w1s", bufs=6))
            w2p = fctx.enter_context(tc.tile_pool(name="w2s", bufs=GF + 2))
            ysp = fctx.enter_context(tc.tile_pool(name="ysb", bufs=2 * TBH))
            rsp = fctx.enter_context(tc.tile_pool(name="rsb", bufs=TBH + 2))

            b1t = fcon.tile([128, FB], F32, tag="b1t")
            nc.sync.dma_start(b1t[:], b1t_d[:])
            b2b = fcon.tile([128, D], F32, tag="b2b")
            nc.sync.dma_start(b2b[:], b2b_d[:])
            bvs = fcon.tile([128, D], F32, tag="bvs")
            nc.sync.dma_start(bvs[:], bvs_d[:])
            lnw = []
            for nm, dd in [("ln1w", ln1w_d), ("ln1b", ln1b_d),
                           ("ln2w", ln2w_d), ("ln2b", ln2b_d)]:
                t = fcon.tile([128, D], F32, tag=nm)
                nc.sync.dma_start(t[:], dd[:])
                lnw.append(t)
            ln1w, ln1b, ln2w, ln2b = lnw
            q1m = fcon.tile([128, SB], F32, tag="q1m")
            nc.sync.dma_start(q1m[:], q1m_d[:])
            qp = fcon.tile([128, SB], F32, tag="qp")
            nc.sync.dma_start(qp[:], qp_d[:])

            def layer_norm(x_ap, w_ap, b_ap, out_ap):
                G = D // 512
                st6 = smalls.tile([128, 6 * G], F32, tag="st6")
                for gg in range(G):
                    nc.vector.bn_stats(st6[:, 6 * gg:6 * gg + 6],
                                       x_ap[:, 512 * gg:512 * gg + 512])
                mv = smalls.tile([128, 2], F32, tag="mv")
                nc.vector.bn_aggr(mv[:], st6[:])
                ve = smalls.tile([128, 1], F32, tag="ve")
                nc.vector.tensor_scalar_add(ve[:], mv[:, 1:2], EPS)
                sd = smalls.tile([128, 1], F32, tag="sd")
                nc.scalar.sqrt(sd[:], ve[:])
                rs_ = smalls.tile([128, 1], F32, tag="rs")
                nc.vector.reciprocal(rs_[:], sd[:])
                xc = stg.tile([128, D], F32, tag="stg")
                nc.vector.tensor_scalar_sub(xc[:], x_ap, mv[:, 0:1])
                nc.vector.scalar_tensor_tensor(out_ap, xc[:], rs_[:], w_ap,
                                               ALU.mult, ALU.mult)
                nc.vector.tensor_add(out_ap, out_ap, b_ap)

            for half in range(HALVES):
                rsrc = rs_outs[half]
                vtr = fcon.tile([1, D], F16, tag="vtr")
                nc.gpsimd.dma_start(vtr[:], rsrc[512:513, :])
                rsbs = []
                for tl in range(TBH):
                    row = tl * 128
                    rsb = rsp.tile([128, D], F16, tag="rsb")
                    nc.gpsimd.dma_start(rsb[:], rsrc[row:row + 128, :])
                    rsbs.append(rsb)
                # broadcast this half's vtot row across partitions
                vtrb = fcon.tile([1, D], BF16, tag="vtrb")
                nc.scalar.copy(vtrb[:], vtr[:])
                vtb = fcon.tile([128, D], F32, tag="vtb")
                for ec in range(EC):
                    bps = pmed.tile([128, 512], F32, tag="med")
                    nc.tensor.matmul(bps[:], ones_bf[0:1, 0:128],
                                     vtrb[0:1, ec * 512:ec * 512 + 512],
                                     start=True, stop=True)
                    nc.scalar.copy(vtb[:, ec * 512:ec * 512 + 512], bps[:])

                x1s = []
                # ---- x0 = blend(attn) + residual; x1 = LN1(x0) ----
                for tl in range(TBH):
                    sblk = half * TBH + tl
                    xsb = stg.tile([128, D], F32, tag="stg")
                    nc.sync.dma_start(xsb[:], xs_d[sblk * 128:sblk * 128 + 128, :])
                    t0 = stg.tile([128, D], F32, tag="stg")
                    nc.vector.scalar_tensor_tensor(
                        t0[:], rsbs[tl][:], q1m[:, sblk:sblk + 1], xsb[:],
                        ALU.mult, ALU.add)
                    t1 = stg.tile([128, D], F32, tag="stg")
                    nc.vector.scalar_tensor_tensor(
                        t1[:], bvs[:], q1m[:, sblk:sblk + 1], t0[:],
                        ALU.mult, ALU.add)
                    x0 = stg.tile([128, D], F32, tag="stg")
                    nc.vector.scalar_tensor_tensor(
                        x0[:], vtb[:], qp[:, sblk:sblk + 1], t1[:],
                        ALU.mult, ALU.add)
                    x1 = x1p.tile([128, D], F32, tag="x1")
                    layer_norm(x0[:], ln1w[:], ln1b[:], x1[:])
                    x1s.append(x1)

                # ---- x1^T (bf16) ----
                x1t = x1tp.tile([128, DC * TPH], BF16, tag="x1t")
                for tl in range(TBH):
                    for c in range(DC):
                        tp = pmed.tile([128, 128], F32, tag="med")
                        nc.tensor.transpose(
                            tp[:], x1s[tl][:, c * 128:c * 128 + 128], ident[:])
                        nc.scalar.copy(
                            x1t[:, c * TPH + tl * 128:c * TPH + tl * 128 + 128],
                            tp[:])

                # ---- hT = relu(W1^T x1^T + b1) (bf16) ----
                hts = htp.tile([128, FB * TPH], BF16, tag="hts")
                for fb in range(FB):
                    w1s = w1p.tile([128, D], BF16, tag="w1s")
                    nc.sync.dma_start(w1s[:], w1_d[fb])
                    hps = pmed.tile([128, TPH], F32, tag="med")
                    for c in range(DC):
                        nc.tensor.matmul(hps[:], w1s[:, c * 128:c * 128 + 128],
                                         x1t[:, c * TPH:(c + 1) * TPH],
                                         start=(c == 0), stop=(c == DC - 1))
                    nc.scalar.activation(hts[:, fb * TPH:(fb + 1) * TPH],
                                         hps[:], AF.Relu,
                                         bias=b1t[:, fb:fb + 1], scale=1.0)

                # ---- y = hT.T @ W2 accumulated over fb groups ----
                ys_prev = [None] * TBH
                NG = FB // GF
                w2cache = {}
                for g in range(NG):
                    for tl in range(TBH):
                        yps = pbig.tile([128, D], F32, tag="big")
                        for j in range(GF):
                            fb = g * GF + j
                            if tl == 0:
                                w2s = w2p.tile([128, D], BF16, tag="w2s")
                                nc.sync.dma_start(
                                    w2s[:], w2_d[fb * 128:fb * 128 + 128, :])
                                w2cache[fb] = w2s
                            w2s = w2cache[fb]
                            for ec in range(EC):
                                nc.tensor.matmul(
                                    yps[:, ec * 512:ec * 512 + 512],
                                    hts[:, fb * TPH + tl * 128:
                                        fb * TPH + tl * 128 + 128],
                                    w2s[:, ec * 512:ec * 512 + 512],
                                    start=(j == 0), stop=(j == GF - 1))
                        ysn = ysp.tile([128, D], F32, tag="ysb")
                        if g == 0:
                            nc.scalar.copy(ysn[:], yps[:])
                        else:
                            nc.vector.scalar_tensor_tensor(
                                ysn[:], yps[:], 1.0, ys_prev[tl][:],
                                ALU.mult, ALU.add)
                        ys_prev[tl] = ysn

                # ---- x2 = x1 + y + b2; out = LN2(x2) ----
                for tl in range(TBH):
                    x2 = stg.tile([128, D], F32, tag="stg")
                    nc.vector.scalar_tensor_tensor(
                        x2[:], ys_prev[tl][:], 1.0, x1s[tl][:],
                        ALU.mult, ALU.add)
                    nc.vector.tensor_add(x2[:], x2[:], b2b[:])
                    ot = stg.tile([128, D], F32, tag="stg")
                    layer_norm(x2[:], ln2w[:], ln2b[:], ot[:])
                    row = (half * TBH + tl) * 128
                    nc.gpsimd.dma_start(out_d[row:row + 128, :], ot[:])

    nc.compile()
    return nc


def _oext_rows(o_exts, spp, b, ic, s, S):
    """Rows [128] of the o_ext tile for query block (b, ic, s).

    spp = slices per RS phase; each 513-row slice = 512 tokens + 1 vmean row.
    """
    grow = b * S + ic * 256 + s * 128
    sl = grow // 512
    row0 = (sl % spp) * 513 + grow % 512
    return o_exts[sl // spp][row0:row0 + 128, :]


# ------------------------- host side -------------------------

_NC_CACHE = {}


def _get_nc(cfg_key):
    if cfg_key not in _NC_CACHE:
        _NC_CACHE[cfg_key] = build_nc(**CFG_MAIN)
    return _NC_CACHE[cfg_key]


def make_in_maps(inputs, B, S, D, FF, ncores, HPC):
    """Build the per-core input dicts from the full (unsharded) inputs."""
    TB = S // 128
    DC = D // 128
    shard = B * S // ncores
    SB = shard // 128
    FB = FF // 128
    H = ncores * HPC
    NS = B * S // 512
    HALVES = NS // ncores
    bf = ml_dtypes.bfloat16
    f8 = ml_dtypes.float8_e4m3

    x = np.ascontiguousarray(
        np.asarray(inputs["input"], dtype=np.float32).reshape(B * S, D))
    xT = np.ascontiguousarray(x.T)
    xTb = xT.astype(bf)
    xT8 = np.clip(xT, -240.0, 240.0).astype(f8)
    pad = np.asarray(inputs["padding_mask"], dtype=bool)
    Wq = np.asarray(inputs["Wq"], dtype=np.float32)
    Wk = np.asarray(inputs["Wk"], dtype=np.float32)
    Wv = np.asarray(inputs["Wv"], dtype=np.float32)
    bq = np.asarray(inputs["bq"], dtype=np.float32)
    bk = np.asarray(inputs["bk"], dtype=np.float32)
    bvv = np.asarray(inputs["bv"], dtype=np.float32)

    padb = np.where(pad, np.float32(NEG), np.float32(0.0)) - np.float32(ESHIFT)
    padb = np.ascontiguousarray(
        padb.reshape(B, TB, 128).transpose(0, 2, 1))

    cmask = np.zeros((128, 128), dtype=np.float32)
    cmask[np.tril_indices(128, -1)] = NEG

    w1 = np.asarray(inputs["ff1_w"], dtype=np.float32)
    w1s = np.ascontiguousarray(
        w1.reshape(D // 128, 128, FB, 128).transpose(2, 1, 0, 3)
        .reshape(FB, 128, D)).astype(bf)
    w2 = np.asarray(inputs["ff2_w"], dtype=np.float32).astype(bf)
    b1 = np.asarray(inputs["ff1_b"], dtype=np.float32)
    b1t = np.ascontiguousarray(b1.reshape(FB, 128).T)
    b2b = np.ascontiguousarray(
        np.broadcast_to(np.asarray(inputs["ff2_b"], np.float32), (128, D)))
    bvs = np.ascontiguousarray(
        np.broadcast_to(bvv.sum(axis=0), (128, D)).astype(np.float32))

    def bc(name):
        return np.ascontiguousarray(np.broadcast_to(
            np.asarray(inputs[name], np.float32), (128, D)))

    ident = np.eye(128, dtype=np.float32)
    padflat = pad.reshape(B * S)

    in_maps = []
    for c in range(ncores):
        h0 = c * HPC
        # wqk[p, w, c, m]: m = (head0 kd 0..63 | head1 kd 0..63)
        wqk = np.empty((128, 2, DC, 128), dtype=np.float32)
        for w, W in ((0, Wq), (1, Wk)):
            for hh in range(HPC):
                Wr = W[h0 + hh].reshape(DC, 128, 64)  # [c, p, kd]
                wqk[:, w, :, hh * 64:(hh + 1) * 64] = Wr.transpose(1, 0, 2)
        wqk = np.ascontiguousarray(wqk.reshape(128, 2 * DC * 128)).astype(bf)
        bqk = np.empty((1, 2 * 128), dtype=np.float32)
        for w, bb in ((0, bq), (1, bk)):
            for hh in range(HPC):
                bqk[0, w * 128 + hh * 64:w * 128 + (hh + 1) * 64] = bb[h0 + hh]
        bqk = bqk.astype(bf)
        # wv8[h, p, c*D + e] = 16*Wv[h, c*128+p, e]
        wv8 = np.ascontiguousarray(
            (Wv[h0:h0 + HPC] * np.float32(WVS))
            .reshape(HPC, DC, 128, D).transpose(0, 2, 1, 3)
            .reshape(HPC, 128, DC * D))
        wv8 = np.clip(wv8, -240.0, 240.0).astype(f8)

        # core c owns slice h*(NS//HALVES)+c for each RS phase h
        tok_idx = np.concatenate([
            np.arange(512 * (h * (NS // HALVES) + c),
                      512 * (h * (NS // HALVES) + c) + 512)
            for h in range(HALVES)])
        prow = padflat[tok_idx].reshape(SB, 128).T
        prow = prow.astype(np.float32)
        m = {
            "xtb": xTb,
            "xt8": xT8,
            "xs": np.ascontiguousarray(x[tok_idx]),
            "wqk": wqk,
            "bqk": bqk,
            "wv8": wv8,
            "bv": np.ascontiguousarray(
                bvv[h0:h0 + HPC].reshape(1, -1)).astype(bf),
            "bvs": bvs,
            "padb": padb,
            "q1m": np.ascontiguousarray((1.0 - prow) / H),
            "qp": np.ascontiguousarray(prow / H),
            "cmask": cmask,
            "ident": ident,
            "w1s": w1s,
            "w2": w2,
            "b1t": b1t,
            "b2b": b2b,
            "ln1w": bc("ln1_w"),
            "ln1b": bc("ln1_b"),
            "ln2w": bc("ln2_w"),
            "ln2b": bc("ln2_b"),
        }
        in_maps.append(m)
    return in_maps


def kernel(**inputs):
    from concourse.bass_utils import run_bass_kernel_spmd
    cfg = CFG_MAIN
    B, S, D = cfg["B"], cfg["S"], cfg["D"]
    ncores = cfg["ncores"]
    shard = B * S // ncores
    nc = _get_nc("main")
    in_maps = make_in_maps(inputs, **cfg)
    res = run_bass_kernel_spmd(nc, in_maps, list(range(ncores)))
    NS = B * S // 512
    HALVES = NS // ncores
    out = np.empty((B * S, D), dtype=np.float32)
    for c in range(ncores):
        r_ = np.asarray(res.results[c]["out"])
        for h in range(HALVES):
            sl = h * (NS // HALVES) + c
            out[512 * sl:512 * sl + 512] = r_[512 * h:512 * h + 512]
    return out.reshape(B, S, D).astype(np.float32)


# revision 16
# speedup vs baseline: 1.3153x; 1.0068x over previous
"""Trainium2 Bass kernel for nn_DecoderLayer (B=4,S=2048,D=1024,H=16,FF=4096).

Sharding: 16 heads / 8 cores = 2 heads per core (tensor/head parallel) for
attention; ReduceScatter (fp16) of the head-summed attention output;
token-parallel LN+FFN on each core's 1/8 token shard; host concatenates.

Numerics: host supplies X^T in bf16 and fp8(e4m3); Q/K projections bf16
(2-head-packed stationary, bias via a K=1 matmul into the same PSUM group);
V projection and P@V run fp8 DoubleRow (2x contraction per instruction) with
exp shifted by -3.5 so softmax numerators fit fp8 range; Wv is scaled x16 on
host so its values sit in fp8 normal range (descaled at V eviction);
softmax/LN in fp32; FFN matmuls bf16; head outputs accumulated in DRAM fp16.
"""
import numpy as np
import ml_dtypes
from contextlib import ExitStack

import concourse.bass as bass
import concourse.tile as tile
from concourse import bacc, mybir

dt = mybir.dt
F32 = dt.float32
F16 = dt.float16
BF16 = dt.bfloat16
FP8 = dt.float8e4
AF = mybir.ActivationFunctionType
ALU = mybir.AluOpType
AX = mybir.AxisListType
DR = mybir.MatmulPerfMode.DoubleRow

KD = 64
EPS = 1e-5
NEG = -30000.0
ESHIFT = 3.5          # exp(score - ESHIFT) keeps numerators in fp8 range
WVS = 16.0            # host scales Wv by x16 into fp8 normal range

CFG_MAIN = dict(B=4, S=2048, D=1024, FF=4096, ncores=8, HPC=2)


def build_nc(B, S, D, FF, ncores, HPC):
    DC = D // 128          # d chunks
    CP = DC // 2           # d-chunk pairs (DoubleRow)
    TB = S // 128          # t blocks per batch
    IC = S // 256          # i chunks (256 queries) per batch
    EC = D // 512          # 512-wide e chunks
    TOKC = S // 512
    shard = B * S // ncores
    SB = shard // 128
    FB = FF // 128
    HALVES = max(1, shard // 512)
    TPH = shard // HALVES  # tokens per FFN half
    TBH = TPH // 128
    GF = 4                 # fb group size for y accumulation
    SPB = S // 512         # 512-token slices per batch
    NS = B * S // 512      # total slices
    assert NS % ncores == 0 and HALVES == NS // ncores

    nc = bacc.Bacc("TRN2", target_bir_lowering=False, debug=False,
                   enable_asserts=False, num_devices=ncores)

    # ---- DRAM I/O ----
    xt_d = nc.dram_tensor("xtb", [D, B * S], BF16, kind="ExternalInput").ap()
    xt8_d = nc.dram_tensor("xt8", [D, B * S], FP8, kind="ExternalInput").ap()
    xs_d = nc.dram_tensor("xs", [shard, D], F32, kind="ExternalInput").ap()
    wqk_d = nc.dram_tensor("wqk", [128, 2 * DC * 128], BF16,
                           kind="ExternalInput").ap()
    bqk_d = nc.dram_tensor("bqk", [1, 2 * 128], BF16, kind="ExternalInput").ap()
    wv8_d = nc.dram_tensor("wv8", [HPC, 128, DC * D], FP8,
                           kind="ExternalInput").ap()
    bv_d = nc.dram_tensor("bv", [1, HPC * D], BF16, kind="ExternalInput").ap()
    bvs_d = nc.dram_tensor("bvs", [128, D], F32, kind="ExternalInput").ap()
    padb_d = nc.dram_tensor("padb", [B, 128, TB], F32, kind="ExternalInput").ap()
    q1m_d = nc.dram_tensor("q1m", [128, SB], F32, kind="ExternalInput").ap()
    qp_d = nc.dram_tensor("qp", [128, SB], F32, kind="ExternalInput").ap()
    cm_d = nc.dram_tensor("cmask", [128, 128], F32, kind="ExternalInput").ap()
    id_d = nc.dram_tensor("ident", [128, 128], F32, kind="ExternalInput").ap()
    w1_d = nc.dram_tensor("w1s", [FB, 128, D], BF16, kind="ExternalInput").ap()
    w2_d = nc.dram_tensor("w2", [FF, D], BF16, kind="ExternalInput").ap()
    b1t_d = nc.dram_tensor("b1t", [128, FB], F32, kind="ExternalInput").ap()
    b2b_d = nc.dram_tensor("b2b", [128, D], F32, kind="ExternalInput").ap()
    ln1w_d = nc.dram_tensor("ln1w", [128, D], F32, kind="ExternalInput").ap()
    ln1b_d = nc.dram_tensor("ln1b", [128, D], F32, kind="ExternalInput").ap()
    ln2w_d = nc.dram_tensor("ln2w", [128, D], F32, kind="ExternalInput").ap()
    ln2b_d = nc.dram_tensor("ln2b", [128, D], F32, kind="ExternalInput").ap()
    out_d = nc.dram_tensor("out", [shard, D], F32, kind="ExternalOutput").ap()

    with tile.TileContext(nc) as tc, ExitStack() as ctx0:
        pbig = ctx0.enter_context(tc.tile_pool(name="pbig", bufs=2, space="PSUM"))
        pmed = ctx0.enter_context(tc.tile_pool(name="pmed", bufs=2, space="PSUM"))
        pdn = ctx0.enter_context(tc.tile_pool(name="pdn", bufs=1, space="PSUM"))
        dramp = ctx0.enter_context(tc.tile_pool(name="dram", bufs=1, space="DRAM"))
        consts = ctx0.enter_context(tc.tile_pool(name="const", bufs=1))
        smalls = ctx0.enter_context(tc.tile_pool(name="smalls", bufs=6))

        o_exts = [dramp.tile([(NS // HALVES) * 513, D], F16,
                             name=f"oext{rp}")
                  for rp in range(HALVES)]
        rs_outs = [dramp.tile([513, D], F16, name=f"rsout{rp}")
                   for rp in range(HALVES)]

        # ---- constants ----
        ident = consts.tile([128, 128], F32, tag="ident")
        nc.sync.dma_start(ident[:], id_d[:])
        cmask = consts.tile([128, 128], F32, tag="cmask")
        nc.sync.dma_start(cmask[:], cm_d[:])
        ones_bf = consts.tile([1, 512], BF16, tag="ones_bf")
        nc.vector.memset(ones_bf[:], 1.0)
        ones8 = consts.tile([128, 32], FP8, tag="ones8")
        nc.vector.memset(ones8[:], 1.0)
        ones8_3 = ones8[:].rearrange("p (j u) -> p j u", u=16)[:, :, 0:1]
        # =================== attention phase ===================
        with ExitStack() as actx:
            xtp = actx.enter_context(tc.tile_pool(name="xt", bufs=2))
            xt8p = actx.enter_context(tc.tile_pool(name="xt8", bufs=2))
            wvp = actx.enter_context(tc.tile_pool(name="wv", bufs=2))
            vp = actx.enter_context(tc.tile_pool(name="v", bufs=2))
            vrp = actx.enter_context(tc.tile_pool(name="vr", bufs=2))
            qkp = actx.enter_context(tc.tile_pool(name="qkt", bufs=2))
            ep = actx.enter_context(tc.tile_pool(name="e", bufs=6))
            osbp = actx.enter_context(tc.tile_pool(name="osb", bufs=4))
            padp = actx.enter_context(tc.tile_pool(name="pad", bufs=3))
            aconp = actx.enter_context(tc.tile_pool(name="acon", bufs=1))

            wqk = aconp.tile([128, 2 * DC * 128], BF16, tag="wqk")
            nc.sync.dma_start(wqk[:], wqk_d[:])
            wqk4 = wqk[:].rearrange("p (w c m) -> p w c m", w=2, c=DC)
            bqk = aconp.tile([1, 2 * 128], BF16, tag="bqk")
            nc.sync.dma_start(bqk[:], bqk_d[:])
            bv = aconp.tile([1, HPC * D], BF16, tag="bv")
            nc.sync.dma_start(bv[:], bv_d[:])

            for b in range(B):
                # ---- load X^T tiles (bf16 for QK, fp8 for V) ----
                xt = xtp.tile([128, DC * S], BF16, tag="xt")
                xt8 = xt8p.tile([128, DC * S], FP8, tag="xt8")
                for c in range(DC):
                    nc.sync.dma_start(
                        xt[:, c * S:(c + 1) * S],
                        xt_d[c * 128:(c + 1) * 128, b * S:(b + 1) * S])
                    nc.sync.dma_start(
                        xt8[:, c * S:(c + 1) * S],
                        xt8_d[c * 128:(c + 1) * 128, b * S:(b + 1) * S])
                xt8_3 = xt8[:].rearrange("p (c t) -> p c t", c=DC)
                # column sums of X (for fully-padded-row vmean)
                xsum = smalls.tile([128, DC], F32, tag="xsum")
                for c in range(DC):
                    nc.vector.tensor_reduce(
                        xsum[:, c:c + 1], xt[:, c * S:(c + 1) * S],
                        AX.X, ALU.add)
                xsum8 = smalls.tile([128, DC], FP8, tag="xsum8")
                nc.vector.tensor_copy(out=xsum8[:], in_=xsum[:])

                padt = padp.tile([128, TB], F32, tag="pad")
                nc.sync.dma_start(padt[:], padb_d[b])

                # ---- Q/K projection, both heads packed (M=128) ----
                qkt2 = qkp.tile([128, 2 * S], BF16, tag="qkt")
                for w in range(2):
                    for c4 in range(TOKC):
                        ps = pmed.tile([128, 512], F32, tag="med")
                        for c in range(DC):
                            nc.tensor.matmul(
                                ps[:], wqk4[:, w, c, :],
                                xt[:, c * S + c4 * 512:c * S + c4 * 512 + 512],
                                start=(c == 0), stop=False)
                        nc.tensor.matmul(
                            ps[:], bqk[0:1, w * 128:w * 128 + 128],
                            ones_bf[0:1, :], start=False, stop=True)
                        off = w * S + c4 * 512
                        nc.scalar.activation(
                            qkt2[:, off:off + 512], ps[:], AF.Copy,
                            scale=(0.125 if w == 0 else 1.0))

                # ---- V projection per head (fp8 DoubleRow) + vmean ----
                v8s = []
                for h in range(HPC):
                    wv8 = wvp.tile([128, DC * D], FP8, tag="wv")
                    nc.sync.dma_start(wv8[:], wv8_d[h])
                    wv8_3 = wv8[:].rearrange("p (c e) -> p c e", c=DC)
                    v8 = vp.tile([128, TB * D], FP8, tag="v", name=f"v8_{h}")
                    for tb in range(TB):
                        vps = pbig.tile([128, D], F32, tag="big")
                        for cc in range(CP):
                            for ec in range(EC):
                                nc.tensor.matmul(
                                    vps[:, ec * 512:ec * 512 + 512],
                                    xt8_3[:, 2 * cc:2 * cc + 2,
                                          tb * 128:tb * 128 + 128],
                                    wv8_3[:, 2 * cc:2 * cc + 2,
                                          ec * 512:ec * 512 + 512],
                                    start=(cc == 0), stop=(cc == CP - 1),
                                    perf_mode=DR)
                        nc.scalar.activation(v8[:, tb * D:tb * D + D], vps[:],
                                             AF.Copy, scale=1.0 / WVS)

                    # vmean (for fully-padded rows), via xsum
                    vmps = [pmed.tile([1, 512], F32, tag="med",
                                      name=f"vmps{ec}")
                            for ec in range(EC)]
                    for ec in range(EC):
                        for c in range(DC):
                            nc.tensor.matmul(
                                vmps[ec][:], xsum8[:, c:c + 1],
                                wv8[:, c * D + ec * 512:c * D + ec * 512 + 512],
                                start=(c == 0), stop=(c == DC - 1))
                    vrow = vrp.tile([1, D], F16, tag="vrow")
                    for ec in range(EC):
                        nc.vector.scalar_tensor_tensor(
                            vrow[0:1, ec * 512:ec * 512 + 512], vmps[ec][:],
                            1.0 / (WVS * S),
                            bv[0:1, h * D + ec * 512:h * D + ec * 512 + 512],
                            ALU.mult, ALU.add)
                    spp = NS // HALVES
                    for k in range(SPB):
                        sl = b * SPB + k
                        vr = (sl % spp) * 513 + 512
                        nc.gpsimd.dma_start(
                            o_exts[sl // spp][vr:vr + 1, :], vrow[0:1, :],
                            accum_op=(ALU.bypass if h == 0 else ALU.add))
                    v8s.append(v8)

                # ---- attention per head (scores bf16, P@V fp8 DoubleRow) ----
                for h in range(HPC):
                    hb = h * 64
                    v8_3 = v8s[h][:].rearrange("p (t e) -> p t e", t=TB)
                    for ic in range(IC):
                        ops = [pbig.tile([128, D], F32, tag="big",
                                         name=f"ops{s}")
                               for s in range(2)]
                        dnps = pdn.tile([1, 256], F32, tag="dn")
                        npair = ic + 1

                        def mk_e(k):
                            e8 = ep.tile([128, 512], FP8, tag="e",
                                         name=f"e{k}")
                            for j in range(2):
                                tb = 2 * k + j
                                st = pmed.tile([128, 256], F32, tag="med",
                                               name=f"st{tb}")
                                nc.tensor.matmul(
                                    st[:],
                                    qkt2[hb:hb + 64,
                                         S + tb * 128:S + tb * 128 + 128],
                                    qkt2[hb:hb + 64,
                                         ic * 256:ic * 256 + 256],
                                    start=True, stop=True)
                                if tb >= 2 * ic:
                                    off = (tb - 2 * ic) * 128
                                    nc.vector.tensor_add(
                                        st[:, off:off + 128],
                                        st[:, off:off + 128], cmask[:])
                                nc.scalar.activation(
                                    e8[:, j * 256:j * 256 + 256], st[:],
                                    AF.Exp, bias=padt[:, tb:tb + 1],
                                    scale=1.0)
                            if k == ic:
                                # pair-last slab, s=0 columns: fully masked
                                nc.vector.memset(e8[:, 256:384], 0.0)
                            return e8

                        epipe = {0: mk_e(0)}
                        if npair > 1:
                            epipe[1] = mk_e(1)
                        for k in range(npair):
                            e8 = epipe.pop(k)
                            if k + 2 < npair:
                                epipe[k + 2] = mk_e(k + 2)
                            e8_3 = e8[:].rearrange("p (j q) -> p j q", j=2)
                            # denominators for both s at once: ones is the
                            # stationary (2-col weight -> near-free LDW)
                            nc.tensor.matmul(
                                dnps[:], ones8_3, e8_3,
                                start=(k == 0), stop=(k == npair - 1),
                                perf_mode=DR)
                            for s in range(2):
                                el = e8_3[:, :, s * 128:s * 128 + 128]
                                for ec in range(EC):
                                    nc.tensor.matmul(
                                        ops[s][:, ec * 512:ec * 512 + 512],
                                        el,
                                        v8_3[:, 2 * k:2 * k + 2,
                                             ec * 512:ec * 512 + 512],
                                        start=(k == 0), stop=(k == npair - 1),
                                        perf_mode=DR)
                        dnb = smalls.tile([1, 256], BF16, tag="dnb")
                        nc.vector.tensor_copy(out=dnb[:], in_=dnps[:])
                        for s in range(2):
                            dtp = pdn.tile([128, 1], F32, tag="dnt")
                            nc.tensor.matmul(
                                dtp[:], dnb[0:1, s * 128:s * 128 + 128],
                                ones_bf[0:1, 0:1], start=True, stop=True)
                            dsb = smalls.tile([128, 1], F32, tag="dsb")
                            nc.vector.tensor_scalar_add(dsb[:], dtp[:],
                                                        1e-30)
                            rd = smalls.tile([128, 1], F32, tag="rd")
                            nc.vector.reciprocal(rd[:], dsb[:])
                            osb = osbp.tile([128, D], F16, tag="osb")
                            nc.vector.tensor_scalar_mul(osb[:], ops[s][:],
                                                        rd[:])
                            nc.gpsimd.dma_start(
                                _oext_rows(o_exts, NS // HALVES, b, ic, s, S),
                                osb[:],
                                accum_op=(ALU.bypass if h == 0 else ALU.add))

                # trigger the RS phase whose batches just completed; the
                # collective runs on the CC cores and overlaps later batches
                bpp = B // HALVES
                if (b + 1) % bpp == 0:
                    rp = (b + 1) // bpp - 1
                    nc.gpsimd.collective_compute(
                        "ReduceScatter", ALU.add,
                        replica_groups=[list(range(ncores))],
                        ins=[o_exts[rp].opt()], outs=[rs_outs[rp].opt()])

        # =================== LN + FFN on the token shard ===================
        with ExitStack() as fctx:
            fcon = fctx.enter_context(tc.tile_pool(name="fcon", bufs=1))
            stg = fctx.enter_context(tc.tile_pool(name="stg", bufs=6))
            x1p = fctx.enter_context(tc.tile_pool(name="x1", bufs=TBH + 2))
            x1tp = fctx.enter_context(tc.tile_pool(name="x1t", bufs=2))
            htp = fctx.enter_context(tc.tile_pool(name="hts", bufs=1))
            w1p = fctx.enter_context(tc.tile_pool(name="w1s", bufs=6))
            w2p = fctx.enter_context(tc.tile_pool(name="w2s", bufs=GF + 2))
            ysp = fctx.enter_context(tc.tile_pool(name="ysb", bufs=2 * TBH))
            rsp = fctx.enter_context(tc.tile_pool(name="rsb", bufs=TBH + 2))

            b1t = fcon.tile([128, FB], F32, tag="b1t")
            nc.sync.dma_start(b1t[:], b1t_d[:])
            b2b = fcon.tile([128, D], F32, tag="b2b")
            nc.sync.dma_start(b2b[:], b2b_d[:])
            bvs = fcon.tile([128, D], F32, tag="bvs")
            nc.sync.dma_start(bvs[:], bvs_d[:])
            lnw = []
            for nm, dd in [("ln1w", ln1w_d), ("ln1b", ln1b_d),
                           ("ln2w", ln2w_d), ("ln2b", ln2b_d)]:
                t = fcon.tile([128, D], F32, tag=nm)
                nc.sync.dma_start(t[:], dd[:])
                lnw.append(t)
            ln1w, ln1b, ln2w, ln2b = lnw
            q1m = fcon.tile([128, SB], F32, tag="q1m")
            nc.sync.dma_start(q1m[:], q1m_d[:])
            qp = fcon.tile([128, SB], F32, tag="qp")
            nc.sync.dma_start(qp[:], qp_d[:])

            def layer_norm(x_ap, w_ap, b_ap, out_ap):
                G = D // 512
                st6 = smalls.tile([128, 6 * G], F32, tag="st6")
                for gg in range(G):
                    nc.vector.bn_stats(st6[:, 6 * gg:6 * gg + 6],
                                       x_ap[:, 512 * gg:512 * gg + 512])
                mv = smalls.tile([128, 2], F32, tag="mv")
                nc.vector.bn_aggr(mv[:], st6[:])
                ve = smalls.tile([128, 1], F32, tag="ve")
                nc.vector.tensor_scalar_add(ve[:], mv[:, 1:2], EPS)
                sd = smalls.tile([128, 1], F32, tag="sd")
                nc.scalar.sqrt(sd[:], ve[:])
                rs_ = smalls.tile([128, 1], F32, tag="rs")
                nc.vector.reciprocal(rs_[:], sd[:])
                xc = stg.tile([128, D], F32, tag="stg")
                nc.vector.tensor_scalar_sub(xc[:], x_ap, mv[:, 0:1])
                nc.vector.scalar_tensor_tensor(out_ap, xc[:], rs_[:], w_ap,
                                               ALU.mult, ALU.mult)
                nc.vector.tensor_add(out_ap, out_ap, b_ap)

            for half in range(HALVES):
                rsrc = rs_outs[half]
                ldq = nc.sync if half == 0 else nc.gpsimd
                vtr = fcon.tile([1, D], F16, tag="vtr")
                ldq.dma_start(vtr[:], rsrc[512:513, :])
                rsbs = []
                for tl in range(TBH):
                    row = tl * 128
                    rsb = rsp.tile([128, D], F16, tag="rsb")
                    ldq.dma_start(rsb[:], rsrc[row:row + 128, :])
                    rsbs.append(rsb)
                # broadcast this half's vtot row across partitions
                vtrb = fcon.tile([1, D], BF16, tag="vtrb")
                nc.vector.tensor_copy(out=vtrb[:], in_=vtr[:])
                vtb = fcon.tile([128, D], F32, tag="vtb")
                for ec in range(EC):
                    bps = pmed.tile([128, 512], F32, tag="med")
                    nc.tensor.matmul(bps[:], ones_bf[0:1, 0:128],
                                     vtrb[0:1, ec * 512:ec * 512 + 512],
                                     start=True, stop=True)
                    nc.scalar.copy(vtb[:, ec * 512:ec * 512 + 512], bps[:])

                x1s = []
                # ---- x0 = blend(attn) + residual; x1 = LN1(x0) ----
                for tl in range(TBH):
                    sblk = half * TBH + tl
                    xsb = stg.tile([128, D], F32, tag="stg")
                    nc.sync.dma_start(xsb[:], xs_d[sblk * 128:sblk * 128 + 128, :])
                    t0 = stg.tile([128, D], F32, tag="stg")
                    nc.vector.scalar_tensor_tensor(
                        t0[:], rsbs[tl][:], q1m[:, sblk:sblk + 1], xsb[:],
                        ALU.mult, ALU.add)
                    t1 = stg.tile([128, D], F32, tag="stg")
                    nc.vector.scalar_tensor_tensor(
                        t1[:], bvs[:], q1m[:, sblk:sblk + 1], t0[:],
                        ALU.mult, ALU.add)
                    x0 = stg.tile([128, D], F32, tag="stg")
                    nc.vector.scalar_tensor_tensor(
                        x0[:], vtb[:], qp[:, sblk:sblk + 1], t1[:],
                        ALU.mult, ALU.add)
                    x1 = x1p.tile([128, D], F32, tag="x1")
                    layer_norm(x0[:], ln1w[:], ln1b[:], x1[:])
                    x1s.append(x1)

                # ---- x1^T (bf16) ----
                x1t = x1tp.tile([128, DC * TPH], BF16, tag="x1t")
                for tl in range(TBH):
                    for c in range(DC):
                        tp = pmed.tile([128, 128], F32, tag="med")
                        nc.tensor.transpose(
                            tp[:], x1s[tl][:, c * 128:c * 128 + 128], ident[:])
                        nc.scalar.copy(
                            x1t[:, c * TPH + tl * 128:c * TPH + tl * 128 + 128],
                            tp[:])

                # ---- hT = relu(W1^T x1^T + b1) (bf16) ----
                hts = htp.tile([128, FB * TPH], BF16, tag="hts")
                for fb in range(FB):
                    w1s = w1p.tile([128, D], BF16, tag="w1s")
                    nc.sync.dma_start(w1s[:], w1_d[fb])
                    hps = pmed.tile([128, TPH], F32, tag="med")
                    for c in range(DC):
                        nc.tensor.matmul(hps[:], w1s[:, c * 128:c * 128 + 128],
                                         x1t[:, c * TPH:(c + 1) * TPH],
                                         start=(c == 0), stop=(c == DC - 1))
                    nc.scalar.activation(hts[:, fb * TPH:(fb + 1) * TPH],
                                         hps[:], AF.Relu,
                                         bias=b1t[:, fb:fb + 1], scale=1.0)

                # ---- y = hT.T @ W2 accumulated over fb groups ----
                ys_prev = [None] * TBH
                NG = FB // GF
                w2cache = {}
                for g in range(NG):
                    for tl in range(TBH):
                        yps = pbig.tile([128, D], F32, tag="big")
                        for j in range(GF):
                            fb = g * GF + j
                            if tl == 0:
                                w2s = w2p.tile([128, D], BF16, tag="w2s")
                                nc.sync.dma_start(
                                    w2s[:], w2_d[fb * 128:fb * 128 + 128, :])
                                w2cache[fb] = w2s
                            w2s = w2cache[fb]
                            for ec in range(EC):
                                nc.tensor.matmul(
                                    yps[:, ec * 512:ec * 512 + 512],
                                    hts[:, fb * TPH + tl * 128:
                                        fb * TPH + tl * 128 + 128],
                                    w2s[:, ec * 512:ec * 512 + 512],
                                    start=(j == 0), stop=(j == GF - 1))
                        ysn = ysp.tile([128, D], F32, tag="ysb")
                        if g == 0:
                            nc.scalar.copy(ysn[:], yps[:])
                        else:
                            nc.vector.scalar_tensor_tensor(
                                ysn[:], yps[:], 1.0, ys_prev[tl][:],
                                ALU.mult, ALU.add)
                        ys_prev[tl] = ysn

                # ---- x2 = x1 + y + b2; out = LN2(x2) ----
                for tl in range(TBH):
                    x2 = stg.tile([128, D], F32, tag="stg")
                    nc.vector.scalar_tensor_tensor(
                        x2[:], ys_prev[tl][:], 1.0, x1s[tl][:],
                        ALU.mult, ALU.add)
                    nc.vector.tensor_add(x2[:], x2[:], b2b[:])
                    ot = stg.tile([128, D], F32, tag="stg")
                    layer_norm(x2[:], ln2w[:], ln2b[:], ot[:])
                    row = (half * TBH + tl) * 128
                    nc.gpsimd.dma_start(out_d[row:row + 128, :], ot[:])

    nc.compile()
    return nc


def _oext_rows(o_exts, spp, b, ic, s, S):
    """Rows [128] of the o_ext tile for query block (b, ic, s).

    spp = slices per RS phase; each 513-row slice = 512 tokens + 1 vmean row.
    """
    grow = b * S + ic * 256 + s * 128
    sl = grow // 512
    row0 = (sl % spp) * 513 + grow % 512
    return o_exts[sl // spp][row0:row0 + 128, :]


# ------------------------- host side -------------------------

_NC_CACHE = {}


def _get_nc(cfg_key):
    if cfg_key not in _NC_CACHE:
        _NC_CACHE[cfg_key] = build_nc(**CFG_MAIN)
    return _NC_CACHE[cfg_key]


def make_in_maps(inputs, B, S, D, FF, ncores, HPC):
    """Build the per-core input dicts from the full (unsharded) inputs."""
    TB = S // 128
    DC = D // 128
    shard = B * S // ncores
    SB = shard // 128
    FB = FF // 128
    H = ncores * HPC
    NS = B * S // 512
    HALVES = NS // ncores
    bf = ml_dtypes.bfloat16
    f8 = ml_dtypes.float8_e4m3

    x = np.ascontiguousarray(
        np.asarray(inputs["input"], dtype=np.float32).reshape(B * S, D))
    xT = np.ascontiguousarray(x.T)
    xTb = xT.astype(bf)
    xT8 = np.clip(xT, -240.0, 240.0).astype(f8)
    pad = np.asarray(inputs["padding_mask"], dtype=bool)
    Wq = np.asarray(inputs["Wq"], dtype=np.float32)
    Wk = np.asarray(inputs["Wk"], dtype=np.float32)
    Wv = np.asarray(inputs["Wv"], dtype=np.float32)
    bq = np.asarray(inputs["bq"], dtype=np.float32)
    bk = np.asarray(inputs["bk"], dtype=np.float32)
    bvv = np.asarray(inputs["bv"], dtype=np.float32)

    padb = np.where(pad, np.float32(NEG), np.float32(0.0)) - np.float32(ESHIFT)
    padb = np.ascontiguousarray(
        padb.reshape(B, TB, 128).transpose(0, 2, 1))

    cmask = np.zeros((128, 128), dtype=np.float32)
    cmask[np.tril_indices(128, -1)] = NEG

    w1 = np.asarray(inputs["ff1_w"], dtype=np.float32)
    w1s = np.ascontiguousarray(
        w1.reshape(D // 128, 128, FB, 128).transpose(2, 1, 0, 3)
        .reshape(FB, 128, D)).astype(bf)
    w2 = np.asarray(inputs["ff2_w"], dtype=np.float32).astype(bf)
    b1 = np.asarray(inputs["ff1_b"], dtype=np.float32)
    b1t = np.ascontiguousarray(b1.reshape(FB, 128).T)
    b2b = np.ascontiguousarray(
        np.broadcast_to(np.asarray(inputs["ff2_b"], np.float32), (128, D)))
    bvs = np.ascontiguousarray(
        np.broadcast_to(bvv.sum(axis=0), (128, D)).astype(np.float32))

    def bc(name):
        return np.ascontiguousarray(np.broadcast_to(
            np.asarray(inputs[name], np.float32), (128, D)))

    ident = np.eye(128, dtype=np.float32)
    padflat = pad.reshape(B * S)

    in_maps = []
    for c in range(ncores):
        h0 = c * HPC
        # wqk[p, w, c, m]: m = (head0 kd 0..63 | head1 kd 0..63)
        wqk = np.empty((128, 2, DC, 128), dtype=np.float32)
        for w, W in ((0, Wq), (1, Wk)):
            for hh in range(HPC):
                Wr = W[h0 + hh].reshape(DC, 128, 64)  # [c, p, kd]
                wqk[:, w, :, hh * 64:(hh + 1) * 64] = Wr.transpose(1, 0, 2)
        wqk = np.ascontiguousarray(wqk.reshape(128, 2 * DC * 128)).astype(bf)
        bqk = np.empty((1, 2 * 128), dtype=np.float32)
        for w, bb in ((0, bq), (1, bk)):
            for hh in range(HPC):
                bqk[0, w * 128 + hh * 64:w * 128 + (hh + 1) * 64] = bb[h0 + hh]
        bqk = bqk.astype(bf)
        # wv8[h, p, c*D + e] = 16*Wv[h, c*128+p, e]
        wv8 = np.ascontiguousarray(
            (Wv[h0:h0 + HPC] * np.float32(WVS))
            .reshape(HPC, DC, 128, D).transpose(0, 2, 1, 3)
            .reshape(HPC, 128, DC * D))
        wv8 = np.clip(wv8, -240.0, 240.0).astype(f8)

        # core c owns slice h*(NS//HALVES)+c for each RS phase h
        tok_idx = np.concatenate([
            np.arange(512 * (h * (NS // HALVES) + c),
                      512 * (h * (NS // HALVES) + c) + 512)
            for h in range(HALVES)])
        prow = padflat[tok_idx].reshape(SB, 128).T
        prow = prow.astype(np.float32)
        m = {
            "xtb": xTb,
            "xt8": xT8,
            "xs": np.ascontiguousarray(x[tok_idx]),
            "wqk": wqk,
            "bqk": bqk,
            "wv8": wv8,
            "bv": np.ascontiguousarray(
                bvv[h0:h0 + HPC].reshape(1, -1)).astype(bf),
            "bvs": bvs,
            "padb": padb,
            "q1m": np.ascontiguousarray((1.0 - prow) / H),
            "qp": np.ascontiguousarray(prow / H),
            "cmask": cmask,
            "ident": ident,
            "w1s": w1s,
            "w2": w2,
            "b1t": b1t,
            "b2b": b2b,
            "ln1w": bc("ln1_w"),
            "ln1b": bc("ln1_b"),
            "ln2w": bc("ln2_w"),
            "ln2b": bc("ln2_b"),
        }
        in_maps.append(m)
    return in_maps


def kernel(**inputs):
    from concourse.bass_utils import run_bass_kernel_spmd
    cfg = CFG_MAIN
    B, S, D = cfg["B"], cfg["S"], cfg["D"]
    ncores = cfg["ncores"]
    shard = B * S // ncores
    nc = _get_nc("main")
    in_maps = make_in_maps(inputs, **cfg)
    res = run_bass_kernel_spmd(nc, in_maps, list(range(ncores)))
    NS = B * S // 512
    HALVES = NS // ncores
    out = np.empty((B * S, D), dtype=np.float32)
    for c in range(ncores):
        r_ = np.asarray(res.results[c]["out"])
        for h in range(HALVES):
            sl = h * (NS // HALVES) + c
            out[512 * sl:512 * sl + 512] = r_[512 * h:512 * h + 512]
    return out.reshape(B, S, D).astype(np.float32)
